# revision 42
# baseline (speedup 1.0000x reference)
"""BMOJO attention (sliding-window + fading memory, joint softmax) on 8 TRN2
NeuronCores via Bass/Tile.

Sharding: tensor-parallel over heads — core c owns q-heads {2c, 2c+1} and kv
head c for both batches and both projection paths; each core computes a partial
output through its Wo column shard and the host sums the 8 partials.

Math (per core, all matmuls bf16 with fp32 PSUM accumulation):
  1. qkv = x @ Wcat.T + bcat  for both paths (Wcat = [Wq_sh; Wk_sh; Wv_sh])
  2. rmsnorm scales r = 1/sqrt(ssq) computed as exp(-0.5*ln(ssq)) so the whole
     kernel uses a single activation table (ln/exp/square/copy); the D**0.25
     rmsnorm/softmax constants are folded into the host-side rope tables
     (cg = cos*g*A, sg = sign*shift(g)*sin*A); rq applied to q, rk folded into
     the exp() scale of the score pass.
  3. scores computed transposed sT[j, i] = k~ @ q~.T so the softmax exp tiles
     feed the PV matmul as the stationary operand without any p-transpose.
     Max-free softmax: p = exp(rk*s), 0/1 block masks after exp.
  4. PV in [i, e] with a ones-column appended to V: the PSUM accumulator picks
     up the joint (in-window + fading) softmax denominator for free.
  5. attn normalized, PE-transposed, then attnT @ WoT_shard -> partial out.

Schedule: stage1 is software-pipelined (chunk tt's PE transposes are emitted
after chunk tt+1's projection matmuls, hiding the ~2.5us rmsnorm/rope
stats latency); batch-0 attention interleaves the remaining batch-1 stage1
chunks and early Wo chunks as PE filler; batch-1 attention interleaves the
rest of the output projection.
"""
import numpy as np
import ml_dtypes

import concourse.bass as bass
import concourse.tile as tile
from concourse import bacc, mybir
from concourse import bass_utils
from concourse.masks import make_identity

BFNP = ml_dtypes.bfloat16
F32 = mybir.dt.float32
BF16 = mybir.dt.bfloat16

B, S, DM = 2, 1024, 2048
H, HKV, D = 16, 8, 128
W = 256
SCALE = D ** -0.5
P = 128
T = B * S           # 2048 flattened tokens
NT = T // P         # 16 t-chunks
ND = DM // P        # 16 d-chunks
NB = S // P         # 8 s-blocks per batch
N_CORES = 8

AluOp = mybir.AluOpType
ACT_FN = mybir.ActivationFunctionType


def _ic_width(bj):
    # in-window scores for key block bj cover query blocks {bj, bj+1, bj+2}
    return min(P * (bj + 3), S) - P * bj


def _f_width(bj):
    # fading scores for key block bj cover query blocks {bj+2 .. NB-1}
    return max(0, S - P * (bj + 2))


def _p1c(tt):
    # dead fading-path columns: its q is never used by queries i < W
    # (s-blocks 0,1) and its k/v never serve keys j > S-W (s-blocks 6,7)
    sblk = tt % NB
    return (256, 512) if sblk <= 1 else (0, 256) if sblk >= NB - 2 \
        else (0, 512)


def _build_tile_kernel(tc):
    nc = tc.nc

    xTt = nc.dram_tensor("xTt", (NT, P, ND, P), BF16, kind="ExternalInput").ap()
    w_ap = [
        nc.dram_tensor(f"wcat{p}", (P, ND, 512), BF16, kind="ExternalInput").ap()
        for p in range(2)
    ]
    b_ap = [
        nc.dram_tensor(f"bcat{p}", (1, 512), BF16, kind="ExternalInput").ap()
        for p in range(2)
    ]
    # rope tables: [sblk, p, path, cg/sg, 384]; identical for both batches
    tabs = nc.dram_tensor("tabs", (NB, P, 2, 2, 384), BF16, kind="ExternalInput").ap()
    wo = nc.dram_tensor("woT", (P, 2, DM), BF16, kind="ExternalInput").ap()
    out = nc.dram_tensor("out", (T, DM), BF16, kind="ExternalOutput").ap()

    consts = tc.alloc_tile_pool(name="consts", bufs=1)
    weights = tc.alloc_tile_pool(name="weights", bufs=1)
    resident = tc.alloc_tile_pool(name="resident", bufs=1)
    xstream = tc.alloc_tile_pool(name="xstream", bufs=7)
    tstream = tc.alloc_tile_pool(name="tstream", bufs=7)
    work = tc.alloc_tile_pool(name="work", bufs=7)
    qpipe = tc.alloc_tile_pool(name="qpipe", bufs=6)
    stats = tc.alloc_tile_pool(name="stats", bufs=4)
    expool = tc.alloc_tile_pool(name="expool", bufs=1)
    outsb = tc.alloc_tile_pool(name="outsb", bufs=3)
    psum_proj = tc.alloc_tile_pool(name="psum_proj", bufs=2, space="PSUM")
    psum_sc = tc.alloc_tile_pool(name="psum_sc", bufs=3, space="PSUM")
    psum_pv = tc.alloc_tile_pool(name="psum_pv", bufs=2, space="PSUM")
    psum_tr = tc.alloc_tile_pool(name="psum_tr", bufs=1, space="PSUM")

    # constants
    ident = consts.tile([P, P], BF16)
    make_identity(nc, ident)
    t1m = consts.tile([P, P], BF16)   # keep i' >= j'  (partition = j', free = i')
    nc.gpsimd.memset(t1m, 1.0)
    nc.gpsimd.affine_select(out=t1m, in_=t1m, compare_op=AluOp.is_ge, fill=0.0,
                            base=0, pattern=[[1, P]], channel_multiplier=-1)
    t2m = consts.tile([P, P], BF16)   # keep i' < j'  i.e. (j' - i' - 1) >= 0
    nc.gpsimd.memset(t2m, 1.0)
    nc.gpsimd.affine_select(out=t2m, in_=t2m, compare_op=AluOp.is_ge, fill=0.0,
                            base=-1, pattern=[[-1, P]], channel_multiplier=1)
    ones1 = consts.tile([1, P], BF16)
    nc.vector.memset(ones1, 1.0)

    # big resident inputs
    wsb = [weights.tile([P, ND, 512], BF16, name=f"wsb{p}") for p in range(2)]
    bsb = [weights.tile([1, 512], BF16, name=f"bsb{p}") for p in range(2)]
    wosb = weights.tile([P, 2, DM], BF16)

    # per-path residents: qkT[path]: [d=128, slot(q0,q1,k), t], v(+ones)
    qkT = [resident.tile([P, 3, T], BF16, name=f"qkT{p}") for p in range(2)]
    vsb = [resident.tile([P, NT, P + 1], BF16, name=f"vsb{p}") for p in range(2)]
    # rall[:, tt, p*3:p*3+3] = (rq0, rq1, rk) = 1/sqrt(ssq) for chunk tt, path p
    rall = resident.tile([P, NT, 6], F32, name="rall")
    for p in range(2):
        nc.vector.memset(vsb[p][:, :, P:P + 1], 1.0)
    attnT = [resident.tile([P, T], BF16, name=f"attnT{h}") for h in range(2)]

    # ---------------- stage 1 (pipelined): mm / consume / finish ------------
    xt_tiles = {}
    tab_tiles = {}

    def prefetch_xt(tt, split=False):
        # split: two pieces so the first d-chunk matmuls can start while the
        # second half is still in flight
        xt = xstream.tile([P, ND, P], BF16, tag="xt")
        if split:
            nc.sync.dma_start(out=xt[:, 0:8], in_=xTt[tt][:, 0:8])
            nc.sync.dma_start(out=xt[:, 8:], in_=xTt[tt][:, 8:])
        else:
            nc.sync.dma_start(out=xt, in_=xTt[tt])
        xt_tiles[tt] = xt

    def prefetch_tab(tt):
        tab = tstream.tile([P, 2, 2, 384], BF16, tag="tab")
        nc.sync.dma_start(out=tab, in_=tabs[tt % NB])
        tab_tiles[tt] = tab

    def prefetch(tt):
        if tt >= NT or tt in xt_tiles:
            return
        prefetch_xt(tt)
        prefetch_tab(tt)

    def stage1_mm(tt, p):
        if p == 0:
            prefetch(tt + 2)
        xt = xt_tiles[tt]
        tab = tab_tiles[tt]
        p1c = _p1c(tt)
        c0, c1 = (0, 512) if p == 0 else p1c
        ps = psum_proj.tile([P, 512], F32, tag="proj")
        # bias via K=1 matmul, then accumulate the 16 d-chunks
        nc.tensor.matmul(ps[:, c0:c1], lhsT=ones1, rhs=bsb[p][:, c0:c1],
                         start=True, stop=False)
        for dd in range(ND):
            nc.tensor.matmul(ps[:, c0:c1], lhsT=xt[:, dd, :],
                             rhs=wsb[p][:, dd, c0:c1],
                             start=False, stop=(dd == ND - 1))
        if p == 1:
            xt_tiles.pop(tt)
        return (tt, p, p1c, ps, tab)

    def stage1_consume(mm_ctx):
        # per-(chunk, path) unit: square/v-copy (Act) and rope (DVE) free the
        # proj psum early; the reduce/Newton/til chain has two whole mm-phases
        # of slack before stage1_finish needs til
        tt, p, p1c, ps, tab = mm_ctx
        c0, sc1 = (0, 384) if p == 0 else (p1c[0], min(p1c[1], 384))
        w = sc1 - c0
        s0, s1 = c0 // P, sc1 // P
        # squares (one ACT op; DVE can't — walrus allows only one PSUM
        # input per instruction)
        sqsb = work.tile([P, 3, P], BF16, tag="sqsb")
        nc.scalar.activation(
            out=sqsb[:, s0:s1, :].rearrange("p a b -> p (a b)"),
            in_=ps[:, c0:sc1], func=ACT_FN.Square)
        # v (+ ones col already set)
        if p == 0 or p1c[1] == 512:
            nc.scalar.copy(out=vsb[p][:, tt, 0:P], in_=ps[:, 384:512])

        # rope over the live head-slots at once
        cg = tab[:, p, 0, c0:sc1]
        sg = tab[:, p, 1, c0:sc1]
        ra = work.tile([P, 384], BF16, tag="ra")
        nc.vector.tensor_tensor(out=ra[:, c0:sc1], in0=ps[:, c0:sc1],
                                in1=cg, op=AluOp.mult)
        # rotate-half read of the psum q/k: one op via a reversed-half AP
        psw = ps[:, c0:sc1]
        pr_sw = bass.AP(tensor=psw.tensor, offset=psw.offset + 64,
                        ap=[list(psw.ap[0]), [128, w // P], [-64, 2],
                            [1, 64]])
        rb = work.tile([P, 384], BF16, tag="rb")
        nc.vector.tensor_tensor(
            out=rb[:, c0:sc1].rearrange("p (h s d) -> p h s d",
                                        h=w // P, s=2, d=64),
            in0=pr_sw,
            in1=sg.rearrange("p (h s d) -> p h s d", h=w // P, s=2, d=64),
            op=AluOp.mult)
        qkn = qpipe.tile([P, 384], BF16, tag="qkn")
        nc.vector.tensor_add(out=qkn[:, c0:sc1], in0=ra[:, c0:sc1],
                             in1=rb[:, c0:sc1])

        # per-head-slot sums (one DVE reduce), then r = 1/sqrt(ssq) via
        # bit-trick + 2 Newton steps, all on DVE ALUs so the Activation
        # engine only ever needs one function table (exp); the D**0.25
        # constants live in the host-folded rope tables and eps is
        # negligible (ssq ~ D >> eps). MAGIC - (x>>1) is computed as
        # ((x>>1) ^ -1) + (MAGIC+1) to avoid a reversed subtract.
        ssq3 = stats.tile([P, 3], F32, tag="ssq3")
        yc = stats.tile([P, 3], F32, tag="yc")
        nt = stats.tile([P, 3], F32, tag="nt")
        xi = ssq3.bitcast(mybir.dt.int32)
        yi = yc.bitcast(mybir.dt.int32)
        nc.vector.tensor_reduce(out=ssq3[:, s0:s1], in_=sqsb[:, s0:s1, :],
                                axis=mybir.AxisListType.X, op=AluOp.add)
        nc.vector.tensor_scalar(out=yi[:, s0:s1], in0=xi[:, s0:s1],
                                scalar1=1, scalar2=-1,
                                op0=AluOp.arith_shift_right,
                                op1=AluOp.bitwise_xor)
        nc.vector.tensor_scalar(out=yi[:, s0:s1], in0=yi[:, s0:s1],
                                scalar1=0x5f3759df + 1, scalar2=None,
                                op0=AluOp.add)
        for it in range(2):
            dst = yc[:, s0:s1] if it == 0 \
                else rall[:, tt, p * 3 + s0:p * 3 + s1]
            nc.vector.tensor_tensor(out=nt[:, s0:s1], in0=yc[:, s0:s1],
                                    in1=yc[:, s0:s1], op=AluOp.mult)
            nc.vector.tensor_tensor(out=nt[:, s0:s1], in0=nt[:, s0:s1],
                                    in1=ssq3[:, s0:s1], op=AluOp.mult)
            nc.vector.tensor_scalar(out=nt[:, s0:s1], in0=nt[:, s0:s1],
                                    scalar1=-0.5, scalar2=1.5,
                                    op0=AluOp.mult, op1=AluOp.add)
            nc.vector.tensor_tensor(out=dst, in0=yc[:, s0:s1],
                                    in1=nt[:, s0:s1], op=AluOp.mult)

        til = None
        if s0 == 0:
            til = qpipe.tile([P, 256], BF16, tag="til")
            for h in range(2):
                nc.vector.tensor_scalar_mul(
                    out=til[:, h * P:(h + 1) * P],
                    in0=qkn[:, h * P:(h + 1) * P],
                    scalar1=rall[:, tt, p * 3 + h:p * 3 + h + 1])
        return (tt, p, qkn, til, c0, sc1)

    def stage1_finish(cons_ctx):
        # emitted two units behind the projection matmuls so the PE-side
        # transposes never wait on the rmsnorm/rope stats chain
        tt, p, qkn, til, c0, sc1 = cons_ctx
        w = sc1 - c0
        s0, s1 = c0 // P, sc1 // P
        tr = psum_sc.tile([P, 384], BF16, tag="sc", name=f"tr{tt}_{p}")
        if s0 == 0:
            nc.tensor.transpose(tr[:, 0:P], til[:, 0:P], ident)
            nc.tensor.transpose(tr[:, P:2 * P], til[:, P:2 * P], ident)
        if s1 == 3:
            nc.tensor.transpose(tr[:, 2 * P:3 * P], qkn[:, 2 * P:3 * P], ident)
        nc.vector.tensor_copy(
            out=qkT[p][:, s0:s1, tt * P:(tt + 1) * P],
            in_=tr[:, c0:sc1].rearrange("p (h t) -> p h t", h=w // P))

    # ---------------- stage 2: attention, both heads of one batch ----------
    # block-level software pipeline: scores/exp/diag-mask of block bj+1 are
    # emitted before the PVs of block bj, so a PV's exp tiles are always a
    # full block-slot old when the PE reaches them
    def stage2_scores(b, bj, exp_ic, exp_f):
        kt_ic = qkT[0][:, 2, (b * S + bj * P):(b * S + (bj + 1) * P)]
        kt_f = qkT[1][:, 2, (b * S + bj * P):(b * S + (bj + 1) * P)]
        w_ic = _ic_width(bj)
        i0 = b * S + bj * P
        wf = _f_width(bj)
        i0f = b * S + P * (bj + 2)
        for h in range(2):
            pssc = psum_sc.tile([P, 512], F32, tag="sc")
            nc.tensor.matmul(pssc[:, 0:w_ic], lhsT=kt_ic,
                             rhs=qkT[0][:, h, i0:i0 + w_ic],
                             start=True, stop=True)
            nc.scalar.activation(out=exp_ic[:, h, bj, 0:w_ic],
                                 in_=pssc[:, 0:w_ic], func=ACT_FN.Exp,
                                 scale=rall[:, b * NB + bj, 2:3])
            for c0 in range(0, wf, 512):
                wc = min(512, wf - c0)
                psf = psum_sc.tile([P, 512], F32, tag="sc")
                nc.tensor.matmul(psf[:, 0:wc], lhsT=kt_f,
                                 rhs=qkT[1][:, h, i0f + c0:i0f + c0 + wc],
                                 start=True, stop=True)
                nc.scalar.activation(
                    out=exp_f[:, h, bj, c0:c0 + wc], in_=psf[:, 0:wc],
                    func=ACT_FN.Exp, scale=rall[:, b * NB + bj, 5:6])
            # only the diagonal mask gates this block's own PV — emit it
            # immediately; the other masks are needed two blocks later
            dia = exp_ic[:, h, bj, 0:P]
            nc.gpsimd.tensor_tensor(out=dia, in0=dia, in1=t1m, op=AluOp.mult)

    def stage2_pv(b, bj, exp_ic, exp_f, group_tr, solo=False):
        w_ic = _ic_width(bj)
        wf = _f_width(bj)
        # PV for query block bi == bj; diagonal (freshest exp) last
        bi = bj
        for h in range(2):
            pv = psum_pv.tile([P, P + 1], F32, tag="pv")
            mms = []
            for bjj in range(0, bi - 1):
                mms.append((exp_f[:, h, bjj, (bi - bjj - 2) * P:(bi - bjj - 1) * P],
                            vsb[1][:, b * NB + bjj, :]))
            for bjj in range(max(0, bi - 2), bi):
                mms.append((exp_ic[:, h, bjj, (bi - bjj) * P:(bi - bjj + 1) * P],
                            vsb[0][:, b * NB + bjj, :]))
            mms.append((exp_ic[:, h, bi, 0:P], vsb[0][:, b * NB + bi, :]))
            for mi, (lhsT, rhs) in enumerate(mms):
                nc.tensor.matmul(pv, lhsT=lhsT, rhs=rhs,
                                 start=(mi == 0), stop=(mi == len(mms) - 1))
            rl = stats.tile([P, 1], F32, tag="rl")
            nc.vector.reciprocal(rl, pv[:, P:P + 1])
            anorm = work.tile([P, P], BF16, tag="anorm")
            nc.vector.tensor_scalar_mul(out=anorm, in0=pv[:, 0:P], scalar1=rl)
            # pair up transposed blocks per psum bank; one copy per pair, so
            # attnT[2b:2b+2] is available to stage3 right after block 2b+1.
            # solo: per-block copy so the final Wo chunks start a block early
            if solo:
                nc.tensor.transpose(group_tr[h][:, 0:P], anorm, ident)
                nc.vector.tensor_copy(
                    out=attnT[h][:, (b * S + bi * P):(b * S + (bi + 1) * P)],
                    in_=group_tr[h][:, 0:P])
                continue
            nc.tensor.transpose(
                group_tr[h][:, (bi % 2) * P:(bi % 2 + 1) * P], anorm, ident)
            if bi % 2 == 1:
                t0 = b * S + (bi - 1) * P
                nc.vector.tensor_copy(out=attnT[h][:, t0:t0 + 256],
                                      in_=group_tr[h])

        # deferred masks (consumed by PV of block bj+2)
        for h in range(2):
            if w_ic > 256:
                ic2 = exp_ic[:, h, bj, 256:384]
                nc.gpsimd.tensor_tensor(out=ic2, in0=ic2, in1=t2m,
                                        op=AluOp.mult)
            if wf > 0:
                f2 = exp_f[:, h, bj, 0:P]
                nc.gpsimd.tensor_tensor(out=f2, in0=f2, in1=t1m,
                                        op=AluOp.mult)

    # ---------------- stage 3: output projection ---------------------------
    def stage3(tt_range, copy_engine="dve", pool=None, last=False):
        for tt in tt_range:
            ot = outsb.tile([P, DM], BF16, tag="ot")
            for oo in range(4):
                if pool is None:
                    po = psum_proj.tile([P, 512], F32, tag="proj")
                elif pool == "alt":
                    if oo % 2 == 0:
                        po = psum_proj.tile([P, 512], F32, tag="proj")
                    else:
                        po = psum_sc.tile([P, 512], F32, tag="sc",
                                          name=f"po{tt}_{oo}")
                else:
                    po = pool.tile([P, 512], F32, tag="sc", name=f"po{tt}_{oo}")
                for h in range(2):
                    nc.tensor.matmul(po, lhsT=attnT[h][:, tt * P:(tt + 1) * P],
                                     rhs=wosb[:, h, oo * 512:(oo + 1) * 512],
                                     start=(h == 0), stop=(h == 1))
                oslice = ot[:, oo * 512:(oo + 1) * 512]
                if last:
                    # drain tail: alternate copy engines and DMA per piece so
                    # copies and output DMAs pipeline instead of serializing
                    if oo % 2 == 0:
                        nc.vector.tensor_copy(out=oslice, in_=po)
                    else:
                        nc.scalar.copy(out=oslice, in_=po)
                    nc.sync.dma_start(
                        out=out[tt * P:(tt + 1) * P, oo * 512:(oo + 1) * 512],
                        in_=oslice)
                    continue
                if copy_engine == "dve":
                    nc.vector.tensor_copy(out=oslice, in_=po)
                elif copy_engine == "act":
                    nc.scalar.copy(out=oslice, in_=po)
                else:  # both
                    if oo % 2 == 0:
                        nc.vector.tensor_copy(out=oslice, in_=po)
                    else:
                        nc.scalar.copy(out=oslice, in_=po)
            if not last:
                # one batched DMA per chunk: 4x fewer HWDGE descriptor setups
                nc.sync.dma_start(out=out[tt * P:(tt + 1) * P, :], in_=ot)

    # ---- emission order tuned for overlap ---------------------------------
    # unit pipeline driver: mm(unit k) | consume(unit k-1) | finish(unit k-2)
    mm_pend = []
    cons_pend = []

    def pump(u=None):
        if u is not None:
            mm_pend.append(stage1_mm(*u))
        if mm_pend and (len(mm_pend) >= 2 or u is None):
            cons_pend.append(stage1_consume(mm_pend.pop(0)))
        if cons_pend and (len(cons_pend) >= 2 or u is None):
            stage1_finish(cons_pend.pop(0))

    # startup DMAs in strict first-use order; path-0 units for chunks 0-3 run
    # first (they only need wcat0 + small x pieces), the fading path starts
    # once its kv weight half lands, its q half streams later still
    nc.sync.dma_start(out=bsb[0], in_=b_ap[0])
    nc.sync.dma_start(out=bsb[1], in_=b_ap[1])
    xt0 = xstream.tile([P, ND, P], BF16, tag="xt")
    nc.sync.dma_start(out=xt0[:, 0:4], in_=xTt[0][:, 0:4])
    nc.sync.dma_start(out=wsb[0][:, 0:4], in_=w_ap[0][:, 0:4])
    nc.sync.dma_start(out=xt0[:, 4:], in_=xTt[0][:, 4:])
    nc.sync.dma_start(out=wsb[0][:, 4:8], in_=w_ap[0][:, 4:8])
    nc.sync.dma_start(out=wsb[0][:, 8:], in_=w_ap[0][:, 8:])
    xt_tiles[0] = xt0
    prefetch_xt(1)
    prefetch_tab(0)
    prefetch_xt(2)
    prefetch_tab(1)
    prefetch_xt(3)
    prefetch_tab(2)
    nc.sync.dma_start(out=wsb[1][:, :, 256:], in_=w_ap[1][:, :, 256:])
    prefetch_tab(3)
    prefetch_xt(4)
    prefetch_tab(4)

    pump((0, 0))
    pump((1, 0))
    pump((2, 0))
    nc.sync.dma_start(out=wsb[1][:, :, 0:256], in_=w_ap[1][:, :, 0:256])
    pump((3, 0))          # prefetches chunk 5
    pump((0, 1))
    pump((4, 0))          # prefetches chunk 6
    nc.sync.dma_start(out=wosb, in_=wo)  # off the startup critical path
    for u in [(1, 1), (5, 0), (2, 1), (6, 0), (3, 1), (7, 0), (4, 1),
              (8, 0), (5, 1), (6, 1), (7, 1), (8, 1)]:
        pump(u)
    pump()   # consume (8,1), finish (7,1): batch-0 qkT complete

    # batch-0 attention, software-pipelined with batch-1 projections and the
    # first Wo chunks as dense PE filler between exp-gated score/PV bursts
    exp0_ic = expool.tile([P, 2, NB, 384], BF16, tag="exp_ic")
    exp0_f = expool.tile([P, 2, 6, 768], BF16, tag="exp_f")
    gtr0_t = psum_tr.tile([P, 512], BF16, tag="gtr")
    gtr0 = [gtr0_t[:, h * 256:(h + 1) * 256] for h in range(2)]
    s3_after0 = {4: [0], 5: [1, 2], 6: [3, 4], 7: [5]}
    stage2_scores(0, 0, exp0_ic, exp0_f)
    for bj in range(NB):
        if bj + 1 < NB:
            stage2_scores(0, bj + 1, exp0_ic, exp0_f)
        if bj == 7:
            # drain the last stage1 units before the final PV so batch-1
            # qkT copies overlap batch-0's tail instead of stalling batch 1
            pump()
            pump()
        stage2_pv(0, bj, exp0_ic, exp0_f, gtr0)
        if bj < 7:
            pump((9 + bj, 0))
            pump((9 + bj, 1))
        for tt in s3_after0.get(bj, []):
            stage3([tt], copy_engine="both", pool=psum_sc)

    # batch-1 attention, with its Wo chunks as filler (bi done at bj >= bi)
    exp1_ic = expool.tile([P, 2, NB, 384], BF16, tag="exp_ic")
    exp1_f = expool.tile([P, 2, 6, 768], BF16, tag="exp_f")
    gtr1_t = psum_tr.tile([P, 512], BF16, tag="gtr")
    gtr1 = [gtr1_t[:, h * 256:(h + 1) * 256] for h in range(2)]
    s3_after1 = {0: [6, 7], 1: [8], 2: [9], 3: [10], 4: [11], 5: [12, 13],
                 6: [14], 7: [15]}
    stage2_scores(1, 0, exp1_ic, exp1_f)
    for bj in range(NB):
        if bj + 1 < NB:
            stage2_scores(1, bj + 1, exp1_ic, exp1_f)
        stage2_pv(1, bj, exp1_ic, exp1_f, gtr1, solo=(bj >= 6))
        for tt in s3_after1.get(bj, []):
            stage3([tt],
                   copy_engine=("dve" if bj <= 3 else "both"),
                   pool=(psum_sc if bj == 7 else None),
                   last=(tt >= NT - 2))

    for pool in reversed((consts, weights, resident, xstream, tstream, work,
                          qpipe, stats, expool, outsb, psum_proj, psum_sc,
                          psum_pv, psum_tr)):
        pool.release()


_NC_CACHE = {}


def _get_nc():
    if "nc" not in _NC_CACHE:
        nc = bacc.Bacc("TRN2", target_bir_lowering=False, debug=False,
                       num_devices=N_CORES)
        with tile.TileContext(nc) as tc:
            _build_tile_kernel(tc)
        nc.compile()
        _NC_CACHE["nc"] = nc
    return _NC_CACHE["nc"]


def _prep_in_maps(inputs):
    f32 = np.float32
    x = np.asarray(inputs["hidden_states"], f32).reshape(T, DM)
    cos = np.asarray(inputs["cos"], f32).reshape(T, D)[:S]
    sin = np.asarray(inputs["sin"], f32).reshape(T, D)[:S]

    xT = np.ascontiguousarray(x.T)
    xTt = np.ascontiguousarray(
        xT.reshape(ND, P, NT, P).transpose(2, 1, 0, 3)).astype(BFNP)

    sign = np.concatenate([-np.ones(64, f32), np.ones(64, f32)])
    A = D ** 0.25   # a*b = SCALE*D split evenly between the q and k tables

    def fold(g):
        g = np.asarray(g, f32)
        cg = cos * (A * g)[None, :]
        sg = sin * (A * sign * np.concatenate([g[64:], g[:64]]))[None, :]
        return cg, sg

    # tabs identical for every core (gammas are global) and both batches
    tabs = np.empty((S, 2, 2, 384), f32)
    for p, (gq_name, gk_name) in enumerate([("gq", "gk"), ("gq2", "gk2")]):
        cgq, sgq = fold(inputs[gq_name])
        cgk, sgk = fold(inputs[gk_name])
        tabs[:, p, 0, :] = np.concatenate([cgq, cgq, cgk], 1)
        tabs[:, p, 1, :] = np.concatenate([sgq, sgq, sgk], 1)
    tabs = tabs.reshape(NB, P, 2, 2, 384).astype(BFNP)

    Wo = np.asarray(inputs["Wo"], f32)

    in_maps = []
    for c in range(N_CORES):
        m = {"xTt": xTt, "tabs": tabs}
        for p, names in enumerate([("Wq", "bq", "Wk", "bk", "Wv", "bv"),
                                   ("Wq2", "bq2", "Wk2", "bk2", "Wv2", "bv2")]):
            Wq, bq, Wk, bk, Wv, bv = (np.asarray(inputs[n], f32) for n in names)
            Wcat = np.concatenate([Wq[c * 256:(c + 1) * 256],
                                   Wk[c * P:(c + 1) * P],
                                   Wv[c * P:(c + 1) * P]], 0)      # [512, DM]
            wcatT = np.ascontiguousarray(Wcat.T)                    # [DM, 512]
            m[f"wcat{p}"] = np.ascontiguousarray(
                wcatT.reshape(ND, P, 512).transpose(1, 0, 2)).astype(BFNP)
            bcat = np.concatenate([bq[c * 256:(c + 1) * 256],
                                   bk[c * P:(c + 1) * P],
                                   bv[c * P:(c + 1) * P]])
            m[f"bcat{p}"] = bcat.reshape(1, 512).astype(BFNP)
        woT = np.ascontiguousarray(Wo[:, c * 256:(c + 1) * 256].T)  # [256, DM]
        m["woT"] = np.ascontiguousarray(
            woT.reshape(2, P, DM).transpose(1, 0, 2)).astype(BFNP)
        in_maps.append(m)
    return in_maps


def kernel(**inputs) -> np.ndarray:
    nc = _get_nc()
    in_maps = _prep_in_maps(inputs)
    res = bass_utils.run_bass_kernel_spmd(nc, in_maps, core_ids=list(range(N_CORES)))
    total = np.zeros((T, DM), np.float32)
    for c in range(N_CORES):
        total += res.results[c]["out"].astype(np.float32)
    return total.reshape(B, S, DM)


# revision 47
# speedup vs baseline: 1.0424x; 1.0424x over previous
"""BMOJO attention (sliding-window + fading memory, joint softmax) on 8 TRN2
NeuronCores via Bass/Tile.

Sharding: tensor-parallel over heads — core c owns q-heads {2c, 2c+1} and kv
head c for both batches and both projection paths; each core computes a partial
output through its Wo column shard and the host sums the 8 partials.

Math (per core, all matmuls bf16 with fp32 PSUM accumulation):
  1. qkv = x @ Wcat.T + bcat  for both paths (Wcat = [Wq_sh; Wk_sh; Wv_sh])
  2. rmsnorm scales r = 1/sqrt(ssq) computed as exp(-0.5*ln(ssq)) so the whole
     kernel uses a single activation table (ln/exp/square/copy); the D**0.25
     rmsnorm/softmax constants are folded into the host-side rope tables
     (cg = cos*g*A, sg = sign*shift(g)*sin*A); rq applied to q, rk folded into
     the exp() scale of the score pass.
  3. scores computed transposed sT[j, i] = k~ @ q~.T so the softmax exp tiles
     feed the PV matmul as the stationary operand without any p-transpose.
     Max-free softmax: p = exp(rk*s), 0/1 block masks after exp.
  4. PV in [i, e] with a ones-column appended to V: the PSUM accumulator picks
     up the joint (in-window + fading) softmax denominator for free.
  5. attn normalized, PE-transposed, then attnT @ WoT_shard -> partial out.

Schedule: stage1 is software-pipelined (chunk tt's PE transposes are emitted
after chunk tt+1's projection matmuls, hiding the ~2.5us rmsnorm/rope
stats latency); batch-0 attention interleaves the remaining batch-1 stage1
chunks and early Wo chunks as PE filler; batch-1 attention interleaves the
rest of the output projection.
"""
import numpy as np
import ml_dtypes

import concourse.bass as bass
import concourse.tile as tile
from concourse import bacc, mybir
from concourse import bass_utils
from concourse.masks import make_identity

BFNP = ml_dtypes.bfloat16
F32 = mybir.dt.float32
BF16 = mybir.dt.bfloat16

B, S, DM = 2, 1024, 2048
H, HKV, D = 16, 8, 128
W = 256
SCALE = D ** -0.5
P = 128
T = B * S           # 2048 flattened tokens
NT = T // P         # 16 t-chunks
ND = DM // P        # 16 d-chunks
NB = S // P         # 8 s-blocks per batch
N_CORES = 8

AluOp = mybir.AluOpType
ACT_FN = mybir.ActivationFunctionType


def _ic_width(bj):
    # in-window scores for key block bj cover query blocks {bj, bj+1, bj+2}
    return min(P * (bj + 3), S) - P * bj


def _f_width(bj):
    # fading scores for key block bj cover query blocks {bj+2 .. NB-1}
    return max(0, S - P * (bj + 2))


def _p1c(tt):
    # dead fading-path columns: its q is never used by queries i < W
    # (s-blocks 0,1) and its k/v never serve keys j > S-W (s-blocks 6,7)
    sblk = tt % NB
    return (256, 512) if sblk <= 1 else (0, 256) if sblk >= NB - 2 \
        else (0, 512)


def _build_tile_kernel(tc):
    nc = tc.nc

    xTt = nc.dram_tensor("xTt", (NT, P, ND, P), BF16, kind="ExternalInput").ap()
    w_ap = [
        nc.dram_tensor(f"wcat{p}", (P, ND, 512), BF16, kind="ExternalInput").ap()
        for p in range(2)
    ]
    b_ap = [
        nc.dram_tensor(f"bcat{p}", (1, 512), BF16, kind="ExternalInput").ap()
        for p in range(2)
    ]
    # rope tables: [sblk, p, path, cg/sg, 384]; identical for both batches
    tabs = nc.dram_tensor("tabs", (NB, P, 2, 2, 384), BF16, kind="ExternalInput").ap()
    wo = nc.dram_tensor("woT", (P, 2, DM), BF16, kind="ExternalInput").ap()
    out = nc.dram_tensor("out", (T, DM), BF16, kind="ExternalOutput").ap()

    consts = tc.alloc_tile_pool(name="consts", bufs=1)
    weights = tc.alloc_tile_pool(name="weights", bufs=1)
    resident = tc.alloc_tile_pool(name="resident", bufs=1)
    xstream = tc.alloc_tile_pool(name="xstream", bufs=7)
    tstream = tc.alloc_tile_pool(name="tstream", bufs=7)
    work = tc.alloc_tile_pool(name="work", bufs=7)
    qpipe = tc.alloc_tile_pool(name="qpipe", bufs=6)
    stats = tc.alloc_tile_pool(name="stats", bufs=4)
    expool = tc.alloc_tile_pool(name="expool", bufs=1)
    outsb = tc.alloc_tile_pool(name="outsb", bufs=3)
    psum_proj = tc.alloc_tile_pool(name="psum_proj", bufs=2, space="PSUM")
    psum_sc = tc.alloc_tile_pool(name="psum_sc", bufs=3, space="PSUM")
    psum_pv = tc.alloc_tile_pool(name="psum_pv", bufs=2, space="PSUM")
    psum_tr = tc.alloc_tile_pool(name="psum_tr", bufs=1, space="PSUM")

    # constants
    ident = consts.tile([P, P], BF16)
    make_identity(nc, ident)
    t1m = consts.tile([P, P], BF16)   # keep i' >= j'  (partition = j', free = i')
    nc.gpsimd.memset(t1m, 1.0)
    nc.gpsimd.affine_select(out=t1m, in_=t1m, compare_op=AluOp.is_ge, fill=0.0,
                            base=0, pattern=[[1, P]], channel_multiplier=-1)
    t2m = consts.tile([P, P], BF16)   # keep i' < j'  i.e. (j' - i' - 1) >= 0
    nc.gpsimd.memset(t2m, 1.0)
    nc.gpsimd.affine_select(out=t2m, in_=t2m, compare_op=AluOp.is_ge, fill=0.0,
                            base=-1, pattern=[[-1, P]], channel_multiplier=1)
    ones1 = consts.tile([1, P], BF16)
    nc.vector.memset(ones1, 1.0)

    # big resident inputs
    wsb = [weights.tile([P, ND, 512], BF16, name=f"wsb{p}") for p in range(2)]
    bsb = [weights.tile([1, 512], BF16, name=f"bsb{p}") for p in range(2)]
    wosb = weights.tile([P, 2, DM], BF16)

    # per-path residents: qkT[path]: [d=128, slot(q0,q1,k), t], v(+ones)
    qkT = [resident.tile([P, 3, T], BF16, name=f"qkT{p}") for p in range(2)]
    vsb = [resident.tile([P, NT, P + 1], BF16, name=f"vsb{p}") for p in range(2)]
    # rall[:, tt, p*3:p*3+3] = (rq0, rq1, rk) = 1/sqrt(ssq) for chunk tt, path p
    rall = resident.tile([P, NT, 6], F32, name="rall")
    for p in range(2):
        nc.vector.memset(vsb[p][:, :, P:P + 1], 1.0)
    attnT = [resident.tile([P, T], BF16, name=f"attnT{h}") for h in range(2)]

    # ---------------- stage 1 (pipelined): mm / consume / finish ------------
    xt_tiles = {}
    tab_tiles = {}

    def prefetch_xt(tt, split=False):
        # split: two pieces so the first d-chunk matmuls can start while the
        # second half is still in flight
        xt = xstream.tile([P, ND, P], BF16, tag="xt")
        if split:
            nc.sync.dma_start(out=xt[:, 0:8], in_=xTt[tt][:, 0:8])
            nc.sync.dma_start(out=xt[:, 8:], in_=xTt[tt][:, 8:])
        else:
            nc.sync.dma_start(out=xt, in_=xTt[tt])
        xt_tiles[tt] = xt

    def prefetch_tab(tt):
        tab = tstream.tile([P, 2, 2, 384], BF16, tag="tab")
        nc.sync.dma_start(out=tab, in_=tabs[tt % NB])
        tab_tiles[tt] = tab

    def prefetch(tt):
        if tt >= NT or tt in xt_tiles:
            return
        prefetch_xt(tt)
        prefetch_tab(tt)

    def stage1_mm(tt, p):
        if p == 0:
            prefetch(tt + 2)
        xt = xt_tiles[tt]
        tab = tab_tiles[tt]
        p1c = _p1c(tt)
        c0, c1 = (0, 512) if p == 0 else p1c
        ps = psum_proj.tile([P, 512], F32, tag="proj")
        # bias via K=1 matmul, then accumulate the 16 d-chunks
        nc.tensor.matmul(ps[:, c0:c1], lhsT=ones1, rhs=bsb[p][:, c0:c1],
                         start=True, stop=False)
        for dd in range(ND):
            nc.tensor.matmul(ps[:, c0:c1], lhsT=xt[:, dd, :],
                             rhs=wsb[p][:, dd, c0:c1],
                             start=False, stop=(dd == ND - 1))
        if p == 1:
            xt_tiles.pop(tt)
        return (tt, p, p1c, ps, tab)

    def stage1_consume(mm_ctx):
        # per-(chunk, path) unit: square/v-copy (Act) and rope (DVE) free the
        # proj psum early; the reduce/Newton/til chain has two whole mm-phases
        # of slack before stage1_finish needs til
        tt, p, p1c, ps, tab = mm_ctx
        c0, sc1 = (0, 384) if p == 0 else (p1c[0], min(p1c[1], 384))
        w = sc1 - c0
        s0, s1 = c0 // P, sc1 // P
        # squares (one ACT op; DVE can't — walrus allows only one PSUM
        # input per instruction)
        sqsb = work.tile([P, 3, P], BF16, tag="sqsb")
        nc.scalar.activation(
            out=sqsb[:, s0:s1, :].rearrange("p a b -> p (a b)"),
            in_=ps[:, c0:sc1], func=ACT_FN.Square)
        # v (+ ones col already set)
        if p == 0 or p1c[1] == 512:
            nc.scalar.copy(out=vsb[p][:, tt, 0:P], in_=ps[:, 384:512])

        # rope over the live head-slots at once
        cg = tab[:, p, 0, c0:sc1]
        sg = tab[:, p, 1, c0:sc1]
        ra = work.tile([P, 384], BF16, tag="ra")
        nc.vector.tensor_tensor(out=ra[:, c0:sc1], in0=ps[:, c0:sc1],
                                in1=cg, op=AluOp.mult)
        # rotate-half read of the psum q/k: one op via a reversed-half AP
        psw = ps[:, c0:sc1]
        pr_sw = bass.AP(tensor=psw.tensor, offset=psw.offset + 64,
                        ap=[list(psw.ap[0]), [128, w // P], [-64, 2],
                            [1, 64]])
        rb = work.tile([P, 384], BF16, tag="rb")
        nc.vector.tensor_tensor(
            out=rb[:, c0:sc1].rearrange("p (h s d) -> p h s d",
                                        h=w // P, s=2, d=64),
            in0=pr_sw,
            in1=sg.rearrange("p (h s d) -> p h s d", h=w // P, s=2, d=64),
            op=AluOp.mult)
        qkn = qpipe.tile([P, 384], BF16, tag="qkn")
        nc.vector.tensor_add(out=qkn[:, c0:sc1], in0=ra[:, c0:sc1],
                             in1=rb[:, c0:sc1])

        # per-head-slot sums (one DVE reduce), then r = 1/sqrt(ssq) via
        # bit-trick + 2 Newton steps, all on DVE ALUs so the Activation
        # engine only ever needs one function table (exp); the D**0.25
        # constants live in the host-folded rope tables and eps is
        # negligible (ssq ~ D >> eps). MAGIC - (x>>1) is computed as
        # ((x>>1) ^ -1) + (MAGIC+1) to avoid a reversed subtract.
        ssq3 = stats.tile([P, 3], F32, tag="ssq3")
        yc = stats.tile([P, 3], F32, tag="yc")
        nt = stats.tile([P, 3], F32, tag="nt")
        xi = ssq3.bitcast(mybir.dt.int32)
        yi = yc.bitcast(mybir.dt.int32)
        nc.vector.tensor_reduce(out=ssq3[:, s0:s1], in_=sqsb[:, s0:s1, :],
                                axis=mybir.AxisListType.X, op=AluOp.add)
        nc.vector.tensor_scalar(out=yi[:, s0:s1], in0=xi[:, s0:s1],
                                scalar1=1, scalar2=-1,
                                op0=AluOp.arith_shift_right,
                                op1=AluOp.bitwise_xor)
        nc.vector.tensor_scalar(out=yi[:, s0:s1], in0=yi[:, s0:s1],
                                scalar1=0x5f3759df + 1, scalar2=None,
                                op0=AluOp.add)
        for it in range(2):
            dst = yc[:, s0:s1] if it == 0 \
                else rall[:, tt, p * 3 + s0:p * 3 + s1]
            nc.vector.tensor_tensor(out=nt[:, s0:s1], in0=yc[:, s0:s1],
                                    in1=yc[:, s0:s1], op=AluOp.mult)
            nc.vector.tensor_tensor(out=nt[:, s0:s1], in0=nt[:, s0:s1],
                                    in1=ssq3[:, s0:s1], op=AluOp.mult)
            nc.vector.tensor_scalar(out=nt[:, s0:s1], in0=nt[:, s0:s1],
                                    scalar1=-0.5, scalar2=1.5,
                                    op0=AluOp.mult, op1=AluOp.add)
            nc.vector.tensor_tensor(out=dst, in0=yc[:, s0:s1],
                                    in1=nt[:, s0:s1], op=AluOp.mult)

        til = None
        if s0 == 0:
            til = qpipe.tile([P, 256], BF16, tag="til")
            for h in range(2):
                nc.vector.tensor_scalar_mul(
                    out=til[:, h * P:(h + 1) * P],
                    in0=qkn[:, h * P:(h + 1) * P],
                    scalar1=rall[:, tt, p * 3 + h:p * 3 + h + 1])
        return (tt, p, qkn, til, c0, sc1)

    def stage1_finish(cons_ctx):
        # emitted two units behind the projection matmuls so the PE-side
        # transposes never wait on the rmsnorm/rope stats chain
        tt, p, qkn, til, c0, sc1 = cons_ctx
        w = sc1 - c0
        s0, s1 = c0 // P, sc1 // P
        tr = psum_sc.tile([P, 384], BF16, tag="sc", name=f"tr{tt}_{p}")
        if s0 == 0:
            nc.tensor.transpose(tr[:, 0:P], til[:, 0:P], ident)
            nc.tensor.transpose(tr[:, P:2 * P], til[:, P:2 * P], ident)
        if s1 == 3:
            nc.tensor.transpose(tr[:, 2 * P:3 * P], qkn[:, 2 * P:3 * P], ident)
        nc.vector.tensor_copy(
            out=qkT[p][:, s0:s1, tt * P:(tt + 1) * P],
            in_=tr[:, c0:sc1].rearrange("p (h t) -> p h t", h=w // P))

    # ---------------- stage 2: attention, both heads of one batch ----------
    # block-level software pipeline: scores/exp/diag-mask of block bj+1 are
    # emitted before the PVs of block bj, so a PV's exp tiles are always a
    # full block-slot old when the PE reaches them
    def stage2_scores(b, bj, exp_ic, exp_f):
        kt_ic = qkT[0][:, 2, (b * S + bj * P):(b * S + (bj + 1) * P)]
        kt_f = qkT[1][:, 2, (b * S + bj * P):(b * S + (bj + 1) * P)]
        w_ic = _ic_width(bj)
        i0 = b * S + bj * P
        wf = _f_width(bj)
        i0f = b * S + P * (bj + 2)
        for h in range(2):
            pssc = psum_sc.tile([P, 512], F32, tag="sc")
            nc.tensor.matmul(pssc[:, 0:w_ic], lhsT=kt_ic,
                             rhs=qkT[0][:, h, i0:i0 + w_ic],
                             start=True, stop=True)
            nc.scalar.activation(out=exp_ic[:, h, bj, 0:w_ic],
                                 in_=pssc[:, 0:w_ic], func=ACT_FN.Exp,
                                 scale=rall[:, b * NB + bj, 2:3])
            for c0 in range(0, wf, 512):
                wc = min(512, wf - c0)
                psf = psum_sc.tile([P, 512], F32, tag="sc")
                nc.tensor.matmul(psf[:, 0:wc], lhsT=kt_f,
                                 rhs=qkT[1][:, h, i0f + c0:i0f + c0 + wc],
                                 start=True, stop=True)
                nc.scalar.activation(
                    out=exp_f[:, h, bj, c0:c0 + wc], in_=psf[:, 0:wc],
                    func=ACT_FN.Exp, scale=rall[:, b * NB + bj, 5:6])
            # only the diagonal mask gates this block's own PV — emit it
            # immediately; the other masks are needed two blocks later
            dia = exp_ic[:, h, bj, 0:P]
            nc.gpsimd.tensor_tensor(out=dia, in0=dia, in1=t1m, op=AluOp.mult)

    def stage2_pv(b, bj, exp_ic, exp_f, group_tr, solo=False):
        w_ic = _ic_width(bj)
        wf = _f_width(bj)
        # PV for query block bi == bj; diagonal (freshest exp) last
        bi = bj
        for h in range(2):
            pv = psum_pv.tile([P, P + 1], F32, tag="pv")
            mms = []
            for bjj in range(0, bi - 1):
                mms.append((exp_f[:, h, bjj, (bi - bjj - 2) * P:(bi - bjj - 1) * P],
                            vsb[1][:, b * NB + bjj, :]))
            for bjj in range(max(0, bi - 2), bi):
                mms.append((exp_ic[:, h, bjj, (bi - bjj) * P:(bi - bjj + 1) * P],
                            vsb[0][:, b * NB + bjj, :]))
            mms.append((exp_ic[:, h, bi, 0:P], vsb[0][:, b * NB + bi, :]))
            for mi, (lhsT, rhs) in enumerate(mms):
                nc.tensor.matmul(pv, lhsT=lhsT, rhs=rhs,
                                 start=(mi == 0), stop=(mi == len(mms) - 1))
            rl = stats.tile([P, 1], F32, tag="rl")
            nc.vector.reciprocal(rl, pv[:, P:P + 1])
            anorm = work.tile([P, P], BF16, tag="anorm")
            nc.vector.tensor_scalar_mul(out=anorm, in0=pv[:, 0:P], scalar1=rl)
            # pair up transposed blocks per psum bank; one copy per pair, so
            # attnT[2b:2b+2] is available to stage3 right after block 2b+1.
            # solo: per-block copy so the final Wo chunks start a block early
            if solo:
                nc.tensor.transpose(group_tr[h][:, 0:P], anorm, ident)
                nc.vector.tensor_copy(
                    out=attnT[h][:, (b * S + bi * P):(b * S + (bi + 1) * P)],
                    in_=group_tr[h][:, 0:P])
                continue
            nc.tensor.transpose(
                group_tr[h][:, (bi % 2) * P:(bi % 2 + 1) * P], anorm, ident)
            if bi % 2 == 1:
                t0 = b * S + (bi - 1) * P
                nc.vector.tensor_copy(out=attnT[h][:, t0:t0 + 256],
                                      in_=group_tr[h])

        # deferred masks (consumed by PV of block bj+2)
        for h in range(2):
            if w_ic > 256:
                ic2 = exp_ic[:, h, bj, 256:384]
                nc.gpsimd.tensor_tensor(out=ic2, in0=ic2, in1=t2m,
                                        op=AluOp.mult)
            if wf > 0:
                f2 = exp_f[:, h, bj, 0:P]
                nc.gpsimd.tensor_tensor(out=f2, in0=f2, in1=t1m,
                                        op=AluOp.mult)

    # ---------------- stage 3: output projection ---------------------------
    def stage3(tt_range, copy_engine="dve", pool=None, last=False):
        for tt in tt_range:
            ot = outsb.tile([P, DM], BF16, tag="ot")
            for oo in range(4):
                if pool is None:
                    po = psum_proj.tile([P, 512], F32, tag="proj")
                elif pool == "alt":
                    if oo % 2 == 0:
                        po = psum_proj.tile([P, 512], F32, tag="proj")
                    else:
                        po = psum_sc.tile([P, 512], F32, tag="sc",
                                          name=f"po{tt}_{oo}")
                else:
                    po = pool.tile([P, 512], F32, tag="sc", name=f"po{tt}_{oo}")
                for h in range(2):
                    nc.tensor.matmul(po, lhsT=attnT[h][:, tt * P:(tt + 1) * P],
                                     rhs=wosb[:, h, oo * 512:(oo + 1) * 512],
                                     start=(h == 0), stop=(h == 1))
                oslice = ot[:, oo * 512:(oo + 1) * 512]
                if last:
                    # drain tail: alternate copy engines and DMA per piece so
                    # copies and output DMAs pipeline instead of serializing
                    if oo % 2 == 0:
                        nc.vector.tensor_copy(out=oslice, in_=po)
                    else:
                        nc.scalar.copy(out=oslice, in_=po)
                    nc.sync.dma_start(
                        out=out[tt * P:(tt + 1) * P, oo * 512:(oo + 1) * 512],
                        in_=oslice)
                    continue
                if copy_engine == "dve":
                    nc.vector.tensor_copy(out=oslice, in_=po)
                elif copy_engine == "act":
                    nc.scalar.copy(out=oslice, in_=po)
                else:  # both
                    if oo % 2 == 0:
                        nc.vector.tensor_copy(out=oslice, in_=po)
                    else:
                        nc.scalar.copy(out=oslice, in_=po)
            if not last:
                # one batched DMA per chunk: 4x fewer HWDGE descriptor setups
                nc.sync.dma_start(out=out[tt * P:(tt + 1) * P, :], in_=ot)

    # ---- emission order tuned for overlap ---------------------------------
    # unit pipeline driver: mm(unit k) | consume(unit k-1) | finish(unit k-2)
    mm_pend = []
    cons_pend = []

    def pump(u=None):
        if u is not None:
            mm_pend.append(stage1_mm(*u))
        if mm_pend and (len(mm_pend) >= 2 or u is None):
            cons_pend.append(stage1_consume(mm_pend.pop(0)))
        if cons_pend and (len(cons_pend) >= 2 or u is None):
            stage1_finish(cons_pend.pop(0))

    # startup DMAs in strict first-use order; path-0 units for chunks 0-3 run
    # first (they only need wcat0 + small x pieces), the fading path starts
    # once its kv weight half lands, its q half streams later still
    nc.sync.dma_start(out=bsb[0], in_=b_ap[0])
    nc.sync.dma_start(out=bsb[1], in_=b_ap[1])
    xt0 = xstream.tile([P, ND, P], BF16, tag="xt")
    nc.sync.dma_start(out=xt0[:, 0:4], in_=xTt[0][:, 0:4])
    nc.sync.dma_start(out=wsb[0][:, 0:4], in_=w_ap[0][:, 0:4])
    nc.sync.dma_start(out=xt0[:, 4:], in_=xTt[0][:, 4:])
    nc.sync.dma_start(out=wsb[0][:, 4:8], in_=w_ap[0][:, 4:8])
    nc.sync.dma_start(out=wsb[0][:, 8:], in_=w_ap[0][:, 8:])
    xt_tiles[0] = xt0
    prefetch_xt(1)
    prefetch_tab(0)
    prefetch_xt(2)
    prefetch_tab(1)
    prefetch_xt(3)
    prefetch_tab(2)
    nc.sync.dma_start(out=wsb[1][:, :, 256:], in_=w_ap[1][:, :, 256:])
    prefetch_tab(3)
    prefetch_xt(4)
    prefetch_tab(4)

    pump((0, 0))
    pump((1, 0))
    pump((2, 0))
    nc.sync.dma_start(out=wsb[1][:, :, 0:256], in_=w_ap[1][:, :, 0:256])
    pump((3, 0))          # prefetches chunk 5
    pump((0, 1))
    pump((4, 0))          # prefetches chunk 6
    nc.sync.dma_start(out=wosb, in_=wo)  # off the startup critical path
    for u in [(1, 1), (5, 0), (2, 1), (6, 0), (3, 1), (7, 0), (4, 1),
              (8, 0), (5, 1), (6, 1), (7, 1), (8, 1)]:
        pump(u)
    pump()   # consume (8,1), finish (7,1): batch-0 qkT complete

    # batch-0 attention, software-pipelined with batch-1 projections and the
    # first Wo chunks as dense PE filler between exp-gated score/PV bursts
    exp0_ic = expool.tile([P, 2, NB, 384], BF16, tag="exp_ic")
    exp0_f = expool.tile([P, 2, 6, 768], BF16, tag="exp_f")
    gtr0_t = psum_tr.tile([P, 512], BF16, tag="gtr")
    gtr0 = [gtr0_t[:, h * 256:(h + 1) * 256] for h in range(2)]
    s3_after0 = {4: [0], 5: [1, 2], 6: [3, 4], 7: [5]}
    stage2_scores(0, 0, exp0_ic, exp0_f)
    for bj in range(NB):
        if bj + 1 < NB:
            stage2_scores(0, bj + 1, exp0_ic, exp0_f)
        if bj == 7:
            # drain the last stage1 units before the final PV so batch-1
            # qkT copies overlap batch-0's tail instead of stalling batch 1
            pump()
            pump()
        stage2_pv(0, bj, exp0_ic, exp0_f, gtr0)
        if bj < 7:
            pump((9 + bj, 0))
            pump((9 + bj, 1))
        for tt in s3_after0.get(bj, []):
            stage3([tt], copy_engine="both", pool=psum_sc)

    # batch-1 attention, with its Wo chunks as filler (bi done at bj >= bi)
    exp1_ic = expool.tile([P, 2, NB, 384], BF16, tag="exp_ic")
    exp1_f = expool.tile([P, 2, 6, 768], BF16, tag="exp_f")
    gtr1_t = psum_tr.tile([P, 512], BF16, tag="gtr")
    gtr1 = [gtr1_t[:, h * 256:(h + 1) * 256] for h in range(2)]
    s3_after1 = {0: [6, 7], 1: [8], 2: [9], 3: [10], 4: [11], 5: [12, 13],
                 6: [14], 7: [15]}
    stage2_scores(1, 0, exp1_ic, exp1_f)
    for bj in range(NB):
        if bj + 1 < NB:
            stage2_scores(1, bj + 1, exp1_ic, exp1_f)
        stage2_pv(1, bj, exp1_ic, exp1_f, gtr1, solo=(bj >= 6))
        for tt in s3_after1.get(bj, []):
            stage3([tt],
                   copy_engine=("dve" if bj <= 3 else "both"),
                   pool=(psum_sc if bj == 7 else None),
                   last=(tt >= NT - 2))

    for pool in reversed((consts, weights, resident, xstream, tstream, work,
                          qpipe, stats, expool, outsb, psum_proj, psum_sc,
                          psum_pv, psum_tr)):
        pool.release()


_NC_CACHE = {}


def _get_nc():
    if "nc" not in _NC_CACHE:
        nc = bacc.Bacc("TRN2", target_bir_lowering=False, debug=False,
                       num_devices=N_CORES)
        with tile.TileContext(nc) as tc:
            _build_tile_kernel(tc)
        nc.compile()
        _NC_CACHE["nc"] = nc
    return _NC_CACHE["nc"]


def _prep_in_maps(inputs):
    f32 = np.float32
    x = np.asarray(inputs["hidden_states"], f32).reshape(T, DM)
    cos = np.asarray(inputs["cos"], f32).reshape(T, D)[:S]
    sin = np.asarray(inputs["sin"], f32).reshape(T, D)[:S]

    xT = np.ascontiguousarray(x.T)
    xTt = np.ascontiguousarray(
        xT.reshape(ND, P, NT, P).transpose(2, 1, 0, 3)).astype(BFNP)

    sign = np.concatenate([-np.ones(64, f32), np.ones(64, f32)])
    A = D ** 0.25   # a*b = SCALE*D split evenly between the q and k tables

    def fold(g):
        g = np.asarray(g, f32)
        cg = cos * (A * g)[None, :]
        sg = sin * (A * sign * np.concatenate([g[64:], g[:64]]))[None, :]
        return cg, sg

    # tabs identical for every core (gammas are global) and both batches
    tabs = np.empty((S, 2, 2, 384), f32)
    for p, (gq_name, gk_name) in enumerate([("gq", "gk"), ("gq2", "gk2")]):
        cgq, sgq = fold(inputs[gq_name])
        cgk, sgk = fold(inputs[gk_name])
        tabs[:, p, 0, :] = np.concatenate([cgq, cgq, cgk], 1)
        tabs[:, p, 1, :] = np.concatenate([sgq, sgq, sgk], 1)
    tabs = tabs.reshape(NB, P, 2, 2, 384).astype(BFNP)

    Wo = np.asarray(inputs["Wo"], f32)

    in_maps = []
    for c in range(N_CORES):
        m = {"xTt": xTt, "tabs": tabs}
        for p, names in enumerate([("Wq", "bq", "Wk", "bk", "Wv", "bv"),
                                   ("Wq2", "bq2", "Wk2", "bk2", "Wv2", "bv2")]):
            Wq, bq, Wk, bk, Wv, bv = (np.asarray(inputs[n], f32) for n in names)
            Wcat = np.concatenate([Wq[c * 256:(c + 1) * 256],
                                   Wk[c * P:(c + 1) * P],
                                   Wv[c * P:(c + 1) * P]], 0)      # [512, DM]
            wcatT = np.ascontiguousarray(Wcat.T)                    # [DM, 512]
            m[f"wcat{p}"] = np.ascontiguousarray(
                wcatT.reshape(ND, P, 512).transpose(1, 0, 2)).astype(BFNP)
            bcat = np.concatenate([bq[c * 256:(c + 1) * 256],
                                   bk[c * P:(c + 1) * P],
                                   bv[c * P:(c + 1) * P]])
            m[f"bcat{p}"] = bcat.reshape(1, 512).astype(BFNP)
        woT = np.ascontiguousarray(Wo[:, c * 256:(c + 1) * 256].T)  # [256, DM]
        m["woT"] = np.ascontiguousarray(
            woT.reshape(2, P, DM).transpose(1, 0, 2)).astype(BFNP)
        in_maps.append(m)
    return in_maps


def kernel(**inputs) -> np.ndarray:
    nc = _get_nc()
    in_maps = _prep_in_maps(inputs)
    res = bass_utils.run_bass_kernel_spmd(nc, in_maps, core_ids=list(range(N_CORES)))
    total = np.zeros((T, DM), np.float32)
    for c in range(N_CORES):
        total += res.results[c]["out"].astype(np.float32)
    return total.reshape(B, S, DM)


# revision 53
# speedup vs baseline: 1.0600x; 1.0169x over previous
"""BMOJO attention (sliding-window + fading memory, joint softmax) on 8 TRN2
NeuronCores via Bass/Tile.

Sharding: tensor-parallel over heads — core c owns q-heads {2c, 2c+1} and kv
head c for both batches and both projection paths; each core computes a partial
output through its Wo column shard and the host sums the 8 partials.

Math (per core, all matmuls bf16 with fp32 PSUM accumulation):
  1. qkv = x @ Wcat.T + bcat  for both paths (Wcat = [Wq_sh; Wk_sh; Wv_sh])
  2. rmsnorm scales r = 1/sqrt(ssq) computed as exp(-0.5*ln(ssq)) so the whole
     kernel uses a single activation table (ln/exp/square/copy); the D**0.25
     rmsnorm/softmax constants are folded into the host-side rope tables
     (cg = cos*g*A, sg = sign*shift(g)*sin*A); rq applied to q, rk folded into
     the exp() scale of the score pass.
  3. scores computed transposed sT[j, i] = k~ @ q~.T so the softmax exp tiles
     feed the PV matmul as the stationary operand without any p-transpose.
     Max-free softmax: p = exp(rk*s), 0/1 block masks after exp.
  4. PV in [i, e] with a ones-column appended to V: the PSUM accumulator picks
     up the joint (in-window + fading) softmax denominator for free.
  5. attn normalized, PE-transposed, then attnT @ WoT_shard -> partial out.

Schedule: stage1 is software-pipelined (chunk tt's PE transposes are emitted
after chunk tt+1's projection matmuls, hiding the ~2.5us rmsnorm/rope
stats latency); batch-0 attention interleaves the remaining batch-1 stage1
chunks and early Wo chunks as PE filler; batch-1 attention interleaves the
rest of the output projection.
"""
import numpy as np
import ml_dtypes

import concourse.bass as bass
import concourse.tile as tile
from concourse import bacc, mybir
from concourse import bass_utils
from concourse.masks import make_identity

BFNP = ml_dtypes.bfloat16
F32 = mybir.dt.float32
BF16 = mybir.dt.bfloat16

B, S, DM = 2, 1024, 2048
H, HKV, D = 16, 8, 128
W = 256
SCALE = D ** -0.5
P = 128
T = B * S           # 2048 flattened tokens
NT = T // P         # 16 t-chunks
ND = DM // P        # 16 d-chunks
NB = S // P         # 8 s-blocks per batch
N_CORES = 8

AluOp = mybir.AluOpType
ACT_FN = mybir.ActivationFunctionType


def _ic_width(bj):
    # in-window scores for key block bj cover query blocks {bj, bj+1, bj+2}
    return min(P * (bj + 3), S) - P * bj


def _f_width(bj):
    # fading scores for key block bj cover query blocks {bj+2 .. NB-1}
    return max(0, S - P * (bj + 2))


def _p1c(tt):
    # dead fading-path columns: its q is never used by queries i < W
    # (s-blocks 0,1) and its k/v never serve keys j > S-W (s-blocks 6,7)
    sblk = tt % NB
    return (256, 512) if sblk <= 1 else (0, 256) if sblk >= NB - 2 \
        else (0, 512)


def _build_tile_kernel(tc):
    nc = tc.nc

    xTt = nc.dram_tensor("xTt", (NT, P, ND, P), BF16, kind="ExternalInput").ap()
    w_ap = [
        nc.dram_tensor(f"wcat{p}", (P, ND, 512), BF16, kind="ExternalInput").ap()
        for p in range(2)
    ]
    b_ap = [
        nc.dram_tensor(f"bcat{p}", (1, 512), BF16, kind="ExternalInput").ap()
        for p in range(2)
    ]
    # rope tables: [sblk, p, path, cg/sg, 384]; identical for both batches
    tabs = nc.dram_tensor("tabs", (NB, P, 2, 2, 384), BF16, kind="ExternalInput").ap()
    wo = nc.dram_tensor("woT", (P, 2, DM), BF16, kind="ExternalInput").ap()
    out = nc.dram_tensor("out", (T, DM), BF16, kind="ExternalOutput").ap()

    consts = tc.alloc_tile_pool(name="consts", bufs=1)
    weights = tc.alloc_tile_pool(name="weights", bufs=1)
    resident = tc.alloc_tile_pool(name="resident", bufs=1)
    xstream = tc.alloc_tile_pool(name="xstream", bufs=7)
    tstream = tc.alloc_tile_pool(name="tstream", bufs=7)
    work = tc.alloc_tile_pool(name="work", bufs=7)
    qpipe = tc.alloc_tile_pool(name="qpipe", bufs=6)
    stats = tc.alloc_tile_pool(name="stats", bufs=4)
    expool = tc.alloc_tile_pool(name="expool", bufs=1)
    outsb = tc.alloc_tile_pool(name="outsb", bufs=3)
    psum_proj = tc.alloc_tile_pool(name="psum_proj", bufs=2, space="PSUM")
    psum_sc = tc.alloc_tile_pool(name="psum_sc", bufs=3, space="PSUM")
    psum_pv = tc.alloc_tile_pool(name="psum_pv", bufs=2, space="PSUM")
    psum_tr = tc.alloc_tile_pool(name="psum_tr", bufs=1, space="PSUM")

    # constants
    ident = consts.tile([P, P], BF16)
    make_identity(nc, ident)
    t1m = consts.tile([P, P], BF16)   # keep i' >= j'  (partition = j', free = i')
    nc.gpsimd.memset(t1m, 1.0)
    nc.gpsimd.affine_select(out=t1m, in_=t1m, compare_op=AluOp.is_ge, fill=0.0,
                            base=0, pattern=[[1, P]], channel_multiplier=-1)
    t2m = consts.tile([P, P], BF16)   # keep i' < j'  i.e. (j' - i' - 1) >= 0
    nc.gpsimd.memset(t2m, 1.0)
    nc.gpsimd.affine_select(out=t2m, in_=t2m, compare_op=AluOp.is_ge, fill=0.0,
                            base=-1, pattern=[[-1, P]], channel_multiplier=1)
    ones1 = consts.tile([1, P], BF16)
    nc.vector.memset(ones1, 1.0)

    # big resident inputs
    wsb = [weights.tile([P, ND, 512], BF16, name=f"wsb{p}") for p in range(2)]
    bsb = [weights.tile([1, 512], BF16, name=f"bsb{p}") for p in range(2)]
    wosb = weights.tile([P, 2, DM], BF16)

    # per-path residents: qkT[path]: [d=128, slot(q0,q1,k), t], v(+ones)
    qkT = [resident.tile([P, 3, T], BF16, name=f"qkT{p}") for p in range(2)]
    vsb = [resident.tile([P, NT, P + 1], BF16, name=f"vsb{p}") for p in range(2)]
    # rall[:, tt, p*3:p*3+3] = (rq0, rq1, rk) = 1/sqrt(ssq) for chunk tt, path p
    rall = resident.tile([P, NT, 6], F32, name="rall")
    for p in range(2):
        nc.vector.memset(vsb[p][:, :, P:P + 1], 1.0)
    attnT = [resident.tile([P, T], BF16, name=f"attnT{h}") for h in range(2)]

    # ---------------- stage 1 (pipelined): mm / consume / finish ------------
    xt_tiles = {}
    tab_tiles = {}

    def prefetch_xt(tt, split=False):
        # split: two pieces so the first d-chunk matmuls can start while the
        # second half is still in flight
        xt = xstream.tile([P, ND, P], BF16, tag="xt")
        if split:
            nc.sync.dma_start(out=xt[:, 0:8], in_=xTt[tt][:, 0:8])
            nc.sync.dma_start(out=xt[:, 8:], in_=xTt[tt][:, 8:])
        else:
            nc.sync.dma_start(out=xt, in_=xTt[tt])
        xt_tiles[tt] = xt

    def prefetch_tab(tt):
        tab = tstream.tile([P, 2, 2, 384], BF16, tag="tab")
        nc.sync.dma_start(out=tab, in_=tabs[tt % NB])
        tab_tiles[tt] = tab

    def prefetch(tt):
        if tt >= NT or tt in xt_tiles:
            return
        prefetch_xt(tt)
        prefetch_tab(tt)

    def stage1_mm(tt, p):
        if p == 0:
            prefetch(tt + 2)
        xt = xt_tiles[tt]
        tab = tab_tiles[tt]
        p1c = _p1c(tt)
        c0, c1 = (0, 512) if p == 0 else p1c
        ps = psum_proj.tile([P, 512], F32, tag="proj")
        # bias via K=1 matmul, then accumulate the 16 d-chunks
        nc.tensor.matmul(ps[:, c0:c1], lhsT=ones1, rhs=bsb[p][:, c0:c1],
                         start=True, stop=False)
        for dd in range(ND):
            nc.tensor.matmul(ps[:, c0:c1], lhsT=xt[:, dd, :],
                             rhs=wsb[p][:, dd, c0:c1],
                             start=False, stop=(dd == ND - 1))
        if p == 1:
            xt_tiles.pop(tt)
        return (tt, p, p1c, ps, tab)

    def stage1_consume(mm_ctx):
        # per-(chunk, path) unit: square/v-copy (Act) and rope (DVE) free the
        # proj psum early; the reduce/Newton/til chain has two whole mm-phases
        # of slack before stage1_finish needs til
        tt, p, p1c, ps, tab = mm_ctx
        c0, sc1 = (0, 384) if p == 0 else (p1c[0], min(p1c[1], 384))
        w = sc1 - c0
        s0, s1 = c0 // P, sc1 // P
        # squares (one ACT op; DVE can't — walrus allows only one PSUM
        # input per instruction)
        sqsb = work.tile([P, 3, P], BF16, tag="sqsb")
        nc.scalar.activation(
            out=sqsb[:, s0:s1, :].rearrange("p a b -> p (a b)"),
            in_=ps[:, c0:sc1], func=ACT_FN.Square)
        # v (+ ones col already set)
        if p == 0 or p1c[1] == 512:
            nc.scalar.copy(out=vsb[p][:, tt, 0:P], in_=ps[:, 384:512])

        # rope over the live head-slots at once
        cg = tab[:, p, 0, c0:sc1]
        sg = tab[:, p, 1, c0:sc1]
        ra = work.tile([P, 384], BF16, tag="ra")
        nc.vector.tensor_tensor(out=ra[:, c0:sc1], in0=ps[:, c0:sc1],
                                in1=cg, op=AluOp.mult)
        # rotate-half read of the psum q/k: one op via a reversed-half AP
        psw = ps[:, c0:sc1]
        pr_sw = bass.AP(tensor=psw.tensor, offset=psw.offset + 64,
                        ap=[list(psw.ap[0]), [128, w // P], [-64, 2],
                            [1, 64]])
        rb = work.tile([P, 384], BF16, tag="rb")
        nc.vector.tensor_tensor(
            out=rb[:, c0:sc1].rearrange("p (h s d) -> p h s d",
                                        h=w // P, s=2, d=64),
            in0=pr_sw,
            in1=sg.rearrange("p (h s d) -> p h s d", h=w // P, s=2, d=64),
            op=AluOp.mult)
        qkn = qpipe.tile([P, 384], BF16, tag="qkn")
        nc.vector.tensor_add(out=qkn[:, c0:sc1], in0=ra[:, c0:sc1],
                             in1=rb[:, c0:sc1])

        # per-head-slot sums (one DVE reduce), then r = 1/sqrt(ssq) via
        # bit-trick + 2 Newton steps, all on DVE ALUs so the Activation
        # engine only ever needs one function table (exp); the D**0.25
        # constants live in the host-folded rope tables and eps is
        # negligible (ssq ~ D >> eps). MAGIC - (x>>1) is computed as
        # ((x>>1) ^ -1) + (MAGIC+1) to avoid a reversed subtract.
        ssq3 = stats.tile([P, 3], F32, tag="ssq3")
        yc = stats.tile([P, 3], F32, tag="yc")
        nt = stats.tile([P, 3], F32, tag="nt")
        xi = ssq3.bitcast(mybir.dt.int32)
        yi = yc.bitcast(mybir.dt.int32)
        nc.vector.tensor_reduce(out=ssq3[:, s0:s1], in_=sqsb[:, s0:s1, :],
                                axis=mybir.AxisListType.X, op=AluOp.add)
        nc.vector.tensor_scalar(out=yi[:, s0:s1], in0=xi[:, s0:s1],
                                scalar1=1, scalar2=-1,
                                op0=AluOp.arith_shift_right,
                                op1=AluOp.bitwise_xor)
        nc.vector.tensor_scalar(out=yi[:, s0:s1], in0=yi[:, s0:s1],
                                scalar1=0x5f3759df + 1, scalar2=None,
                                op0=AluOp.add)
        for it in range(2):
            dst = yc[:, s0:s1] if it == 0 \
                else rall[:, tt, p * 3 + s0:p * 3 + s1]
            nc.vector.tensor_tensor(out=nt[:, s0:s1], in0=yc[:, s0:s1],
                                    in1=yc[:, s0:s1], op=AluOp.mult)
            nc.vector.tensor_tensor(out=nt[:, s0:s1], in0=nt[:, s0:s1],
                                    in1=ssq3[:, s0:s1], op=AluOp.mult)
            nc.vector.tensor_scalar(out=nt[:, s0:s1], in0=nt[:, s0:s1],
                                    scalar1=-0.5, scalar2=1.5,
                                    op0=AluOp.mult, op1=AluOp.add)
            nc.vector.tensor_tensor(out=dst, in0=yc[:, s0:s1],
                                    in1=nt[:, s0:s1], op=AluOp.mult)

        til = None
        if s0 == 0:
            # rq scaling on the Act engine (copy with per-partition scale) —
            # DVE is the tight engine during stage1
            til = qpipe.tile([P, 256], BF16, tag="til")
            for h in range(2):
                nc.scalar.activation(
                    out=til[:, h * P:(h + 1) * P],
                    in_=qkn[:, h * P:(h + 1) * P], func=ACT_FN.Copy,
                    scale=rall[:, tt, p * 3 + h:p * 3 + h + 1])
        return (tt, p, qkn, til, c0, sc1)

    def stage1_finish(cons_ctx):
        # emitted two units behind the projection matmuls so the PE-side
        # transposes never wait on the rmsnorm/rope stats chain
        tt, p, qkn, til, c0, sc1 = cons_ctx
        w = sc1 - c0
        s0, s1 = c0 // P, sc1 // P
        tr = psum_sc.tile([P, 384], BF16, tag="sc", name=f"tr{tt}_{p}")
        if s0 == 0:
            nc.tensor.transpose(tr[:, 0:P], til[:, 0:P], ident)
            nc.tensor.transpose(tr[:, P:2 * P], til[:, P:2 * P], ident)
        if s1 == 3:
            nc.tensor.transpose(tr[:, 2 * P:3 * P], qkn[:, 2 * P:3 * P], ident)
        nc.vector.tensor_copy(
            out=qkT[p][:, s0:s1, tt * P:(tt + 1) * P],
            in_=tr[:, c0:sc1].rearrange("p (h t) -> p h t", h=w // P))

    # ---------------- stage 2: attention, both heads of one batch ----------
    # block-level software pipeline: scores/exp/diag-mask of block bj+1 are
    # emitted before the PVs of block bj, so a PV's exp tiles are always a
    # full block-slot old when the PE reaches them
    def stage2_scores(b, bj, exp_ic, exp_f):
        kt_ic = qkT[0][:, 2, (b * S + bj * P):(b * S + (bj + 1) * P)]
        kt_f = qkT[1][:, 2, (b * S + bj * P):(b * S + (bj + 1) * P)]
        w_ic = _ic_width(bj)
        i0 = b * S + bj * P
        wf = _f_width(bj)
        i0f = b * S + P * (bj + 2)
        for h in range(2):
            pssc = psum_sc.tile([P, 512], F32, tag="sc")
            nc.tensor.matmul(pssc[:, 0:w_ic], lhsT=kt_ic,
                             rhs=qkT[0][:, h, i0:i0 + w_ic],
                             start=True, stop=True)
            nc.scalar.activation(out=exp_ic[:, h, bj, 0:w_ic],
                                 in_=pssc[:, 0:w_ic], func=ACT_FN.Exp,
                                 scale=rall[:, b * NB + bj, 2:3])
            for c0 in range(0, wf, 512):
                wc = min(512, wf - c0)
                psf = psum_sc.tile([P, 512], F32, tag="sc")
                nc.tensor.matmul(psf[:, 0:wc], lhsT=kt_f,
                                 rhs=qkT[1][:, h, i0f + c0:i0f + c0 + wc],
                                 start=True, stop=True)
                nc.scalar.activation(
                    out=exp_f[:, h, bj, c0:c0 + wc], in_=psf[:, 0:wc],
                    func=ACT_FN.Exp, scale=rall[:, b * NB + bj, 5:6])
            # only the diagonal mask gates this block's own PV — emit it
            # immediately; the other masks are needed two blocks later
            dia = exp_ic[:, h, bj, 0:P]
            nc.gpsimd.tensor_tensor(out=dia, in0=dia, in1=t1m, op=AluOp.mult)

    def stage2_pv(b, bj, exp_ic, exp_f, group_tr, solo=False):
        w_ic = _ic_width(bj)
        wf = _f_width(bj)
        # PV for query block bi == bj; diagonal (freshest exp) last
        bi = bj
        for h in range(2):
            pv = psum_pv.tile([P, P + 1], F32, tag="pv")
            mms = []
            for bjj in range(0, bi - 1):
                mms.append((exp_f[:, h, bjj, (bi - bjj - 2) * P:(bi - bjj - 1) * P],
                            vsb[1][:, b * NB + bjj, :]))
            for bjj in range(max(0, bi - 2), bi):
                mms.append((exp_ic[:, h, bjj, (bi - bjj) * P:(bi - bjj + 1) * P],
                            vsb[0][:, b * NB + bjj, :]))
            mms.append((exp_ic[:, h, bi, 0:P], vsb[0][:, b * NB + bi, :]))
            for mi, (lhsT, rhs) in enumerate(mms):
                nc.tensor.matmul(pv, lhsT=lhsT, rhs=rhs,
                                 start=(mi == 0), stop=(mi == len(mms) - 1))
            rl = stats.tile([P, 1], F32, tag="rl")
            nc.vector.reciprocal(rl, pv[:, P:P + 1])
            anorm = work.tile([P, P], BF16, tag="anorm")
            nc.vector.tensor_scalar_mul(out=anorm, in0=pv[:, 0:P], scalar1=rl)
            # pair up transposed blocks per psum bank; one copy per pair, so
            # attnT[2b:2b+2] is available to stage3 right after block 2b+1.
            # solo: per-block copy so the final Wo chunks start a block early
            if solo:
                nc.tensor.transpose(group_tr[h][:, 0:P], anorm, ident)
                nc.vector.tensor_copy(
                    out=attnT[h][:, (b * S + bi * P):(b * S + (bi + 1) * P)],
                    in_=group_tr[h][:, 0:P])
                continue
            nc.tensor.transpose(
                group_tr[h][:, (bi % 2) * P:(bi % 2 + 1) * P], anorm, ident)
            if bi % 2 == 1:
                t0 = b * S + (bi - 1) * P
                nc.vector.tensor_copy(out=attnT[h][:, t0:t0 + 256],
                                      in_=group_tr[h])

        # deferred masks (consumed by PV of block bj+2)
        for h in range(2):
            if w_ic > 256:
                ic2 = exp_ic[:, h, bj, 256:384]
                nc.gpsimd.tensor_tensor(out=ic2, in0=ic2, in1=t2m,
                                        op=AluOp.mult)
            if wf > 0:
                f2 = exp_f[:, h, bj, 0:P]
                nc.gpsimd.tensor_tensor(out=f2, in0=f2, in1=t1m,
                                        op=AluOp.mult)

    # ---------------- stage 3: output projection ---------------------------
    def stage3(tt_range, copy_engine="dve", pool=None, last=False):
        for tt in tt_range:
            ot = outsb.tile([P, DM], BF16, tag="ot")
            for oo in range(4):
                if pool is None:
                    po = psum_proj.tile([P, 512], F32, tag="proj")
                elif pool == "alt":
                    if oo % 2 == 0:
                        po = psum_proj.tile([P, 512], F32, tag="proj")
                    else:
                        po = psum_sc.tile([P, 512], F32, tag="sc",
                                          name=f"po{tt}_{oo}")
                else:
                    po = pool.tile([P, 512], F32, tag="sc", name=f"po{tt}_{oo}")
                for h in range(2):
                    nc.tensor.matmul(po, lhsT=attnT[h][:, tt * P:(tt + 1) * P],
                                     rhs=wosb[:, h, oo * 512:(oo + 1) * 512],
                                     start=(h == 0), stop=(h == 1))
                oslice = ot[:, oo * 512:(oo + 1) * 512]
                if last:
                    # drain tail: alternate copy engines and DMA per 1KB-wide
                    # half so copies and output DMAs pipeline instead of
                    # serializing four descriptor setups at the very end
                    if oo % 2 == 0:
                        nc.vector.tensor_copy(out=oslice, in_=po)
                    else:
                        nc.scalar.copy(out=oslice, in_=po)
                        nc.sync.dma_start(
                            out=out[tt * P:(tt + 1) * P,
                                    (oo - 1) * 512:(oo + 1) * 512],
                            in_=ot[:, (oo - 1) * 512:(oo + 1) * 512])
                    continue
                if copy_engine == "dve":
                    nc.vector.tensor_copy(out=oslice, in_=po)
                elif copy_engine == "act":
                    nc.scalar.copy(out=oslice, in_=po)
                else:  # both
                    if oo % 2 == 0:
                        nc.vector.tensor_copy(out=oslice, in_=po)
                    else:
                        nc.scalar.copy(out=oslice, in_=po)
            if not last:
                # one batched DMA per chunk: 4x fewer HWDGE descriptor setups
                nc.sync.dma_start(out=out[tt * P:(tt + 1) * P, :], in_=ot)

    # ---- emission order tuned for overlap ---------------------------------
    # unit pipeline driver: mm(unit k) | consume(unit k-1) | finish(unit k-2)
    mm_pend = []
    cons_pend = []

    def pump(u=None):
        if u is not None:
            mm_pend.append(stage1_mm(*u))
        if mm_pend and (len(mm_pend) >= 2 or u is None):
            cons_pend.append(stage1_consume(mm_pend.pop(0)))
        if cons_pend and (len(cons_pend) >= 2 or u is None):
            stage1_finish(cons_pend.pop(0))

    # startup DMAs in strict first-use order; path-0 units for chunks 0-3 run
    # first (they only need wcat0 + small x pieces), the fading path starts
    # once its kv weight half lands, its q half streams later still
    nc.sync.dma_start(out=bsb[0], in_=b_ap[0])
    nc.sync.dma_start(out=bsb[1], in_=b_ap[1])
    xt0 = xstream.tile([P, ND, P], BF16, tag="xt")
    nc.sync.dma_start(out=xt0[:, 0:4], in_=xTt[0][:, 0:4])
    nc.sync.dma_start(out=wsb[0][:, 0:4], in_=w_ap[0][:, 0:4])
    nc.sync.dma_start(out=xt0[:, 4:], in_=xTt[0][:, 4:])
    nc.sync.dma_start(out=wsb[0][:, 4:8], in_=w_ap[0][:, 4:8])
    nc.sync.dma_start(out=wsb[0][:, 8:], in_=w_ap[0][:, 8:])
    xt_tiles[0] = xt0
    prefetch_xt(1)
    prefetch_tab(0)
    prefetch_xt(2)
    prefetch_tab(1)
    prefetch_xt(3)
    prefetch_tab(2)
    nc.sync.dma_start(out=wsb[1][:, :, 256:], in_=w_ap[1][:, :, 256:])
    prefetch_tab(3)
    prefetch_xt(4)
    prefetch_tab(4)

    pump((0, 0))
    pump((1, 0))
    pump((2, 0))
    nc.sync.dma_start(out=wsb[1][:, :, 0:256], in_=w_ap[1][:, :, 0:256])
    pump((3, 0))          # prefetches chunk 5
    pump((0, 1))
    pump((4, 0))          # prefetches chunk 6
    nc.sync.dma_start(out=wosb, in_=wo)  # off the startup critical path
    for u in [(1, 1), (5, 0), (2, 1), (6, 0), (3, 1), (7, 0), (4, 1),
              (8, 0), (5, 1), (6, 1), (7, 1), (8, 1)]:
        pump(u)
    pump()   # consume (8,1), finish (7,1): batch-0 qkT complete

    # batch-0 attention, software-pipelined with batch-1 projections and the
    # first Wo chunks as dense PE filler between exp-gated score/PV bursts
    exp0_ic = expool.tile([P, 2, NB, 384], BF16, tag="exp_ic")
    exp0_f = expool.tile([P, 2, 6, 768], BF16, tag="exp_f")
    gtr0_t = psum_tr.tile([P, 512], BF16, tag="gtr")
    gtr0 = [gtr0_t[:, h * 256:(h + 1) * 256] for h in range(2)]
    s3_after0 = {4: [0], 5: [1, 2], 6: [3, 4], 7: [5]}
    stage2_scores(0, 0, exp0_ic, exp0_f)
    for bj in range(NB):
        if bj + 1 < NB:
            stage2_scores(0, bj + 1, exp0_ic, exp0_f)
        if bj == 7:
            # drain the last stage1 units before the final PV so batch-1
            # qkT copies overlap batch-0's tail instead of stalling batch 1
            pump()
            pump()
        if bj < 7:
            pump((9 + bj, 0))
        stage2_pv(0, bj, exp0_ic, exp0_f, gtr0)
        if bj < 7:
            pump((9 + bj, 1))
        for tt in s3_after0.get(bj, []):
            stage3([tt], copy_engine="act", pool=psum_sc)

    # batch-1 attention, with its Wo chunks as filler (bi done at bj >= bi)
    exp1_ic = expool.tile([P, 2, NB, 384], BF16, tag="exp_ic")
    exp1_f = expool.tile([P, 2, 6, 768], BF16, tag="exp_f")
    gtr1_t = psum_tr.tile([P, 512], BF16, tag="gtr")
    gtr1 = [gtr1_t[:, h * 256:(h + 1) * 256] for h in range(2)]
    s3_after1 = {0: [6, 7], 1: [8], 2: [9], 3: [10], 4: [11], 5: [12, 13],
                 6: [14], 7: [15]}
    stage2_scores(1, 0, exp1_ic, exp1_f)
    for bj in range(NB):
        if bj + 1 < NB:
            stage2_scores(1, bj + 1, exp1_ic, exp1_f)
        stage2_pv(1, bj, exp1_ic, exp1_f, gtr1, solo=(bj >= 6))
        for tt in s3_after1.get(bj, []):
            stage3([tt],
                   copy_engine=("dve" if bj <= 3 else "both"),
                   pool=(psum_sc if bj == 7 else None),
                   last=(tt >= NT - 2))

    for pool in reversed((consts, weights, resident, xstream, tstream, work,
                          qpipe, stats, expool, outsb, psum_proj, psum_sc,
                          psum_pv, psum_tr)):
        pool.release()


_NC_CACHE = {}


def _get_nc():
    if "nc" not in _NC_CACHE:
        nc = bacc.Bacc("TRN2", target_bir_lowering=False, debug=False,
                       num_devices=N_CORES)
        with tile.TileContext(nc) as tc:
            _build_tile_kernel(tc)
        nc.compile()
        _NC_CACHE["nc"] = nc
    return _NC_CACHE["nc"]


def _prep_in_maps(inputs):
    f32 = np.float32
    x = np.asarray(inputs["hidden_states"], f32).reshape(T, DM)
    cos = np.asarray(inputs["cos"], f32).reshape(T, D)[:S]
    sin = np.asarray(inputs["sin"], f32).reshape(T, D)[:S]

    xT = np.ascontiguousarray(x.T)
    xTt = np.ascontiguousarray(
        xT.reshape(ND, P, NT, P).transpose(2, 1, 0, 3)).astype(BFNP)

    sign = np.concatenate([-np.ones(64, f32), np.ones(64, f32)])
    A = D ** 0.25   # a*b = SCALE*D split evenly between the q and k tables

    def fold(g):
        g = np.asarray(g, f32)
        cg = cos * (A * g)[None, :]
        sg = sin * (A * sign * np.concatenate([g[64:], g[:64]]))[None, :]
        return cg, sg

    # tabs identical for every core (gammas are global) and both batches
    tabs = np.empty((S, 2, 2, 384), f32)
    for p, (gq_name, gk_name) in enumerate([("gq", "gk"), ("gq2", "gk2")]):
        cgq, sgq = fold(inputs[gq_name])
        cgk, sgk = fold(inputs[gk_name])
        tabs[:, p, 0, :] = np.concatenate([cgq, cgq, cgk], 1)
        tabs[:, p, 1, :] = np.concatenate([sgq, sgq, sgk], 1)
    tabs = tabs.reshape(NB, P, 2, 2, 384).astype(BFNP)

    Wo = np.asarray(inputs["Wo"], f32)

    in_maps = []
    for c in range(N_CORES):
        m = {"xTt": xTt, "tabs": tabs}
        for p, names in enumerate([("Wq", "bq", "Wk", "bk", "Wv", "bv"),
                                   ("Wq2", "bq2", "Wk2", "bk2", "Wv2", "bv2")]):
            Wq, bq, Wk, bk, Wv, bv = (np.asarray(inputs[n], f32) for n in names)
            Wcat = np.concatenate([Wq[c * 256:(c + 1) * 256],
                                   Wk[c * P:(c + 1) * P],
                                   Wv[c * P:(c + 1) * P]], 0)      # [512, DM]
            wcatT = np.ascontiguousarray(Wcat.T)                    # [DM, 512]
            m[f"wcat{p}"] = np.ascontiguousarray(
                wcatT.reshape(ND, P, 512).transpose(1, 0, 2)).astype(BFNP)
            bcat = np.concatenate([bq[c * 256:(c + 1) * 256],
                                   bk[c * P:(c + 1) * P],
                                   bv[c * P:(c + 1) * P]])
            m[f"bcat{p}"] = bcat.reshape(1, 512).astype(BFNP)
        woT = np.ascontiguousarray(Wo[:, c * 256:(c + 1) * 256].T)  # [256, DM]
        m["woT"] = np.ascontiguousarray(
            woT.reshape(2, P, DM).transpose(1, 0, 2)).astype(BFNP)
        in_maps.append(m)
    return in_maps


def kernel(**inputs) -> np.ndarray:
    nc = _get_nc()
    in_maps = _prep_in_maps(inputs)
    res = bass_utils.run_bass_kernel_spmd(nc, in_maps, core_ids=list(range(N_CORES)))
    total = np.zeros((T, DM), np.float32)
    for c in range(N_CORES):
        total += res.results[c]["out"].astype(np.float32)
    return total.reshape(B, S, DM)


# revision 63
# speedup vs baseline: 1.0777x; 1.0167x over previous
"""BMOJO attention (sliding-window + fading memory, joint softmax) on 8 TRN2
NeuronCores via Bass/Tile.

Sharding: tensor-parallel over heads — core c owns q-heads {2c, 2c+1} and kv
head c for both batches and both projection paths; each core computes a partial
output through its Wo column shard and the host sums the 8 partials.

Math (per core, all matmuls bf16 with fp32 PSUM accumulation):
  1. qkv = x @ Wcat.T + bcat  for both paths (Wcat = [Wq_sh; Wk_sh; Wv_sh])
  2. rmsnorm scales r = 1/sqrt(ssq) computed as exp(-0.5*ln(ssq)) so the whole
     kernel uses a single activation table (ln/exp/square/copy); the D**0.25
     rmsnorm/softmax constants are folded into the host-side rope tables
     (cg = cos*g*A, sg = sign*shift(g)*sin*A); rq applied to q, rk folded into
     the exp() scale of the score pass.
  3. scores computed transposed sT[j, i] = k~ @ q~.T so the softmax exp tiles
     feed the PV matmul as the stationary operand without any p-transpose.
     Max-free softmax: p = exp(rk*s), 0/1 block masks after exp.
  4. PV in [i, e] with a ones-column appended to V: the PSUM accumulator picks
     up the joint (in-window + fading) softmax denominator for free.
  5. attn normalized, PE-transposed, then attnT @ WoT_shard -> partial out.

Schedule: stage1 is software-pipelined (chunk tt's PE transposes are emitted
after chunk tt+1's projection matmuls, hiding the ~2.5us rmsnorm/rope
stats latency); batch-0 attention interleaves the remaining batch-1 stage1
chunks and early Wo chunks as PE filler; batch-1 attention interleaves the
rest of the output projection.
"""
import numpy as np
import ml_dtypes

import concourse.bass as bass
import concourse.tile as tile
from concourse import bacc, mybir
from concourse import bass_utils
from concourse.masks import make_identity

BFNP = ml_dtypes.bfloat16
F32 = mybir.dt.float32
BF16 = mybir.dt.bfloat16

B, S, DM = 2, 1024, 2048
H, HKV, D = 16, 8, 128
W = 256
SCALE = D ** -0.5
P = 128
T = B * S           # 2048 flattened tokens
NT = T // P         # 16 t-chunks
ND = DM // P        # 16 d-chunks
NB = S // P         # 8 s-blocks per batch
N_CORES = 8

AluOp = mybir.AluOpType
ACT_FN = mybir.ActivationFunctionType


def _ic_width(bj):
    # in-window scores for key block bj cover query blocks {bj, bj+1, bj+2}
    return min(P * (bj + 3), S) - P * bj


def _f_width(bj):
    # fading scores for key block bj cover query blocks {bj+2 .. NB-1}
    return max(0, S - P * (bj + 2))


def _p1c(tt):
    # dead fading-path columns: its q is never used by queries i < W
    # (s-blocks 0,1) and its k/v never serve keys j > S-W (s-blocks 6,7)
    sblk = tt % NB
    return (256, 512) if sblk <= 1 else (0, 256) if sblk >= NB - 2 \
        else (0, 512)


def _build_tile_kernel(tc):
    nc = tc.nc

    xTt = nc.dram_tensor("xTt", (NT, P, ND, P), BF16, kind="ExternalInput").ap()
    w_ap = [
        nc.dram_tensor(f"wcat{p}", (P, ND, 512), BF16, kind="ExternalInput").ap()
        for p in range(2)
    ]
    b_ap = [
        nc.dram_tensor(f"bcat{p}", (1, 512), BF16, kind="ExternalInput").ap()
        for p in range(2)
    ]
    # rope tables: [sblk, p, path, cg/sg, 384]; identical for both batches
    tabs = nc.dram_tensor("tabs", (NB, P, 2, 2, 384), BF16, kind="ExternalInput").ap()
    wo = nc.dram_tensor("woT", (P, 2, DM), BF16, kind="ExternalInput").ap()
    out = nc.dram_tensor("out", (T, DM), BF16, kind="ExternalOutput").ap()

    consts = tc.alloc_tile_pool(name="consts", bufs=1)
    weights = tc.alloc_tile_pool(name="weights", bufs=1)
    resident = tc.alloc_tile_pool(name="resident", bufs=1)
    xstream = tc.alloc_tile_pool(name="xstream", bufs=7)
    tstream = tc.alloc_tile_pool(name="tstream", bufs=7)
    work = tc.alloc_tile_pool(name="work", bufs=7)
    qpipe = tc.alloc_tile_pool(name="qpipe", bufs=6)
    stats = tc.alloc_tile_pool(name="stats", bufs=4)
    expool = tc.alloc_tile_pool(name="expool", bufs=1)
    outsb = tc.alloc_tile_pool(name="outsb", bufs=3)
    psum_proj = tc.alloc_tile_pool(name="psum_proj", bufs=2, space="PSUM")
    psum_sc = tc.alloc_tile_pool(name="psum_sc", bufs=3, space="PSUM")
    psum_pv = tc.alloc_tile_pool(name="psum_pv", bufs=2, space="PSUM")
    psum_tr = tc.alloc_tile_pool(name="psum_tr", bufs=1, space="PSUM")

    # constants
    ident = consts.tile([P, P], BF16)
    make_identity(nc, ident)
    t1m = consts.tile([P, P], BF16)   # keep i' >= j'  (partition = j', free = i')
    nc.gpsimd.memset(t1m, 1.0)
    nc.gpsimd.affine_select(out=t1m, in_=t1m, compare_op=AluOp.is_ge, fill=0.0,
                            base=0, pattern=[[1, P]], channel_multiplier=-1)
    t2m = consts.tile([P, P], BF16)   # keep i' < j'  i.e. (j' - i' - 1) >= 0
    nc.gpsimd.memset(t2m, 1.0)
    nc.gpsimd.affine_select(out=t2m, in_=t2m, compare_op=AluOp.is_ge, fill=0.0,
                            base=-1, pattern=[[-1, P]], channel_multiplier=1)
    ones1 = consts.tile([1, P], BF16)
    nc.vector.memset(ones1, 1.0)

    # big resident inputs
    wsb = [weights.tile([P, ND, 512], BF16, name=f"wsb{p}") for p in range(2)]
    bsb = [weights.tile([1, 512], BF16, name=f"bsb{p}") for p in range(2)]
    wosb = weights.tile([P, 2, DM], BF16)

    # per-path residents: qkT[path]: [d=128, slot(q0,q1,k), t], v(+ones)
    qkT = [resident.tile([P, 3, T], BF16, name=f"qkT{p}") for p in range(2)]
    vsb = [resident.tile([P, NT, P + 1], BF16, name=f"vsb{p}") for p in range(2)]
    # rall[:, tt, p*3:p*3+3] = (rq0, rq1, rk) = 1/sqrt(ssq) for chunk tt, path p
    rall = resident.tile([P, NT, 6], F32, name="rall")
    for p in range(2):
        nc.vector.memset(vsb[p][:, :, P:P + 1], 1.0)
    attnT = [resident.tile([P, T], BF16, name=f"attnT{h}") for h in range(2)]

    # ---------------- stage 1 (pipelined): mm / consume / finish ------------
    xt_tiles = {}
    tab_tiles = {}

    def prefetch_xt(tt, split=False):
        # split: two pieces so the first d-chunk matmuls can start while the
        # second half is still in flight
        xt = xstream.tile([P, ND, P], BF16, tag="xt")
        if split:
            nc.sync.dma_start(out=xt[:, 0:8], in_=xTt[tt][:, 0:8])
            nc.sync.dma_start(out=xt[:, 8:], in_=xTt[tt][:, 8:])
        else:
            nc.sync.dma_start(out=xt, in_=xTt[tt])
        xt_tiles[tt] = xt

    def prefetch_tab(tt):
        tab = tstream.tile([P, 2, 2, 384], BF16, tag="tab")
        nc.sync.dma_start(out=tab, in_=tabs[tt % NB])
        tab_tiles[tt] = tab

    def prefetch(tt):
        if tt >= NT or tt in xt_tiles:
            return
        prefetch_xt(tt)
        prefetch_tab(tt)

    def stage1_mm(tt, p):
        if p == 0:
            prefetch(tt + 2)
        xt = xt_tiles[tt]
        tab = tab_tiles[tt]
        p1c = _p1c(tt)
        c0, c1 = (0, 512) if p == 0 else p1c
        ps = psum_proj.tile([P, 512], F32, tag="proj")
        # bias via K=1 matmul, then accumulate the 16 d-chunks
        nc.tensor.matmul(ps[:, c0:c1], lhsT=ones1, rhs=bsb[p][:, c0:c1],
                         start=True, stop=False)
        for dd in range(ND):
            nc.tensor.matmul(ps[:, c0:c1], lhsT=xt[:, dd, :],
                             rhs=wsb[p][:, dd, c0:c1],
                             start=False, stop=(dd == ND - 1))
        if p == 1:
            xt_tiles.pop(tt)
        return (tt, p, p1c, ps, tab)

    def stage1_consume(mm_ctx):
        # per-(chunk, path) unit: square/v-copy (Act) and rope (DVE) free the
        # proj psum early; the reduce/Newton/til chain has two whole mm-phases
        # of slack before stage1_finish needs til
        tt, p, p1c, ps, tab = mm_ctx
        c0, sc1 = (0, 384) if p == 0 else (p1c[0], min(p1c[1], 384))
        w = sc1 - c0
        s0, s1 = c0 // P, sc1 // P
        # squares (one ACT op; DVE can't — walrus allows only one PSUM
        # input per instruction)
        sqsb = work.tile([P, 3, P], BF16, tag="sqsb")
        nc.scalar.activation(
            out=sqsb[:, s0:s1, :].rearrange("p a b -> p (a b)"),
            in_=ps[:, c0:sc1], func=ACT_FN.Square)
        # v (+ ones col already set)
        if p == 0 or p1c[1] == 512:
            nc.scalar.copy(out=vsb[p][:, tt, 0:P], in_=ps[:, 384:512])

        # rope over the live head-slots at once
        cg = tab[:, p, 0, c0:sc1]
        sg = tab[:, p, 1, c0:sc1]
        ra = work.tile([P, 384], BF16, tag="ra")
        nc.vector.tensor_tensor(out=ra[:, c0:sc1], in0=ps[:, c0:sc1],
                                in1=cg, op=AluOp.mult)
        # rotate-half read of the psum q/k: one op via a reversed-half AP
        psw = ps[:, c0:sc1]
        pr_sw = bass.AP(tensor=psw.tensor, offset=psw.offset + 64,
                        ap=[list(psw.ap[0]), [128, w // P], [-64, 2],
                            [1, 64]])
        rb = work.tile([P, 384], BF16, tag="rb")
        nc.vector.tensor_tensor(
            out=rb[:, c0:sc1].rearrange("p (h s d) -> p h s d",
                                        h=w // P, s=2, d=64),
            in0=pr_sw,
            in1=sg.rearrange("p (h s d) -> p h s d", h=w // P, s=2, d=64),
            op=AluOp.mult)
        qkn = qpipe.tile([P, 384], BF16, tag="qkn")
        nc.vector.tensor_add(out=qkn[:, c0:sc1], in0=ra[:, c0:sc1],
                             in1=rb[:, c0:sc1])

        # per-head-slot sums (one DVE reduce), then r = 1/sqrt(ssq) via
        # bit-trick + 2 Newton steps, all on DVE ALUs so the Activation
        # engine only ever needs one function table (exp); the D**0.25
        # constants live in the host-folded rope tables and eps is
        # negligible (ssq ~ D >> eps). MAGIC - (x>>1) is computed as
        # ((x>>1) ^ -1) + (MAGIC+1) to avoid a reversed subtract.
        ssq3 = stats.tile([P, 3], F32, tag="ssq3")
        yc = stats.tile([P, 3], F32, tag="yc")
        nt = stats.tile([P, 3], F32, tag="nt")
        xi = ssq3.bitcast(mybir.dt.int32)
        yi = yc.bitcast(mybir.dt.int32)
        nc.vector.tensor_reduce(out=ssq3[:, s0:s1], in_=sqsb[:, s0:s1, :],
                                axis=mybir.AxisListType.X, op=AluOp.add)
        nc.vector.tensor_scalar(out=yi[:, s0:s1], in0=xi[:, s0:s1],
                                scalar1=1, scalar2=-1,
                                op0=AluOp.arith_shift_right,
                                op1=AluOp.bitwise_xor)
        nc.vector.tensor_scalar(out=yi[:, s0:s1], in0=yi[:, s0:s1],
                                scalar1=0x5f3759df + 1, scalar2=None,
                                op0=AluOp.add)
        for it in range(2):
            dst = yc[:, s0:s1] if it == 0 \
                else rall[:, tt, p * 3 + s0:p * 3 + s1]
            nc.vector.tensor_tensor(out=nt[:, s0:s1], in0=yc[:, s0:s1],
                                    in1=yc[:, s0:s1], op=AluOp.mult)
            nc.vector.tensor_tensor(out=nt[:, s0:s1], in0=nt[:, s0:s1],
                                    in1=ssq3[:, s0:s1], op=AluOp.mult)
            nc.vector.tensor_scalar(out=nt[:, s0:s1], in0=nt[:, s0:s1],
                                    scalar1=-0.5, scalar2=1.5,
                                    op0=AluOp.mult, op1=AluOp.add)
            nc.vector.tensor_tensor(out=dst, in0=yc[:, s0:s1],
                                    in1=nt[:, s0:s1], op=AluOp.mult)

        til = None
        if s0 == 0:
            # rq scaling on the Act engine (copy with per-partition scale) —
            # DVE is the tight engine during stage1
            til = qpipe.tile([P, 256], BF16, tag="til")
            for h in range(2):
                nc.scalar.activation(
                    out=til[:, h * P:(h + 1) * P],
                    in_=qkn[:, h * P:(h + 1) * P], func=ACT_FN.Copy,
                    scale=rall[:, tt, p * 3 + h:p * 3 + h + 1])
        return (tt, p, qkn, til, c0, sc1)

    def stage1_finish(cons_ctx):
        # emitted two units behind the projection matmuls so the PE-side
        # transposes never wait on the rmsnorm/rope stats chain
        tt, p, qkn, til, c0, sc1 = cons_ctx
        w = sc1 - c0
        s0, s1 = c0 // P, sc1 // P
        tr = psum_sc.tile([P, 384], BF16, tag="sc", name=f"tr{tt}_{p}")
        if s0 == 0:
            nc.tensor.transpose(tr[:, 0:P], til[:, 0:P], ident)
            nc.tensor.transpose(tr[:, P:2 * P], til[:, P:2 * P], ident)
        if s1 == 3:
            nc.tensor.transpose(tr[:, 2 * P:3 * P], qkn[:, 2 * P:3 * P], ident)
        nc.vector.tensor_copy(
            out=qkT[p][:, s0:s1, tt * P:(tt + 1) * P],
            in_=tr[:, c0:sc1].rearrange("p (h t) -> p h t", h=w // P))

    # ---------------- stage 2: attention, both heads of one batch ----------
    # block-level software pipeline: scores/exp/diag-mask of block bj+1 are
    # emitted before the PVs of block bj, so a PV's exp tiles are always a
    # full block-slot old when the PE reaches them
    def stage2_scores(b, bj, exp_ic, exp_f):
        kt_ic = qkT[0][:, 2, (b * S + bj * P):(b * S + (bj + 1) * P)]
        kt_f = qkT[1][:, 2, (b * S + bj * P):(b * S + (bj + 1) * P)]
        w_ic = _ic_width(bj)
        i0 = b * S + bj * P
        wf = _f_width(bj)
        i0f = b * S + P * (bj + 2)
        for h in range(2):
            pssc = psum_sc.tile([P, 512], F32, tag="sc")
            nc.tensor.matmul(pssc[:, 0:w_ic], lhsT=kt_ic,
                             rhs=qkT[0][:, h, i0:i0 + w_ic],
                             start=True, stop=True)
            nc.scalar.activation(out=exp_ic[:, h, bj, 0:w_ic],
                                 in_=pssc[:, 0:w_ic], func=ACT_FN.Exp,
                                 scale=rall[:, b * NB + bj, 2:3])
            for c0 in range(0, wf, 512):
                wc = min(512, wf - c0)
                psf = psum_sc.tile([P, 512], F32, tag="sc")
                nc.tensor.matmul(psf[:, 0:wc], lhsT=kt_f,
                                 rhs=qkT[1][:, h, i0f + c0:i0f + c0 + wc],
                                 start=True, stop=True)
                nc.scalar.activation(
                    out=exp_f[:, h, bj, c0:c0 + wc], in_=psf[:, 0:wc],
                    func=ACT_FN.Exp, scale=rall[:, b * NB + bj, 5:6])
            # only the diagonal mask gates this block's own PV — emit it
            # immediately; the other masks are needed two blocks later
            dia = exp_ic[:, h, bj, 0:P]
            nc.gpsimd.tensor_tensor(out=dia, in0=dia, in1=t1m, op=AluOp.mult)

    def stage2_pv(b, bj, exp_ic, exp_f, group_tr, solo=False):
        w_ic = _ic_width(bj)
        wf = _f_width(bj)
        # PV for query block bi == bj; diagonal (freshest exp) last
        bi = bj
        for h in range(2):
            pv = psum_pv.tile([P, P + 1], F32, tag="pv")
            mms = []
            for bjj in range(0, bi - 1):
                mms.append((exp_f[:, h, bjj, (bi - bjj - 2) * P:(bi - bjj - 1) * P],
                            vsb[1][:, b * NB + bjj, :]))
            for bjj in range(max(0, bi - 2), bi):
                mms.append((exp_ic[:, h, bjj, (bi - bjj) * P:(bi - bjj + 1) * P],
                            vsb[0][:, b * NB + bjj, :]))
            mms.append((exp_ic[:, h, bi, 0:P], vsb[0][:, b * NB + bi, :]))
            for mi, (lhsT, rhs) in enumerate(mms):
                nc.tensor.matmul(pv, lhsT=lhsT, rhs=rhs,
                                 start=(mi == 0), stop=(mi == len(mms) - 1))
            rl = stats.tile([P, 1], F32, tag="rl")
            nc.vector.reciprocal(rl, pv[:, P:P + 1])
            anorm = work.tile([P, P], BF16, tag="anorm")
            nc.vector.tensor_scalar_mul(out=anorm, in0=pv[:, 0:P], scalar1=rl)
            # pair up transposed blocks per psum bank; one copy per pair, so
            # attnT[2b:2b+2] is available to stage3 right after block 2b+1.
            # solo: per-block copy so the final Wo chunks start a block early
            if solo:
                nc.tensor.transpose(group_tr[h][:, 0:P], anorm, ident)
                nc.vector.tensor_copy(
                    out=attnT[h][:, (b * S + bi * P):(b * S + (bi + 1) * P)],
                    in_=group_tr[h][:, 0:P])
                continue
            nc.tensor.transpose(
                group_tr[h][:, (bi % 2) * P:(bi % 2 + 1) * P], anorm, ident)
            if bi % 2 == 1:
                t0 = b * S + (bi - 1) * P
                nc.vector.tensor_copy(out=attnT[h][:, t0:t0 + 256],
                                      in_=group_tr[h])

        # deferred masks (consumed by PV of block bj+2)
        for h in range(2):
            if w_ic > 256:
                ic2 = exp_ic[:, h, bj, 256:384]
                nc.gpsimd.tensor_tensor(out=ic2, in0=ic2, in1=t2m,
                                        op=AluOp.mult)
            if wf > 0:
                f2 = exp_f[:, h, bj, 0:P]
                nc.gpsimd.tensor_tensor(out=f2, in0=f2, in1=t1m,
                                        op=AluOp.mult)

    # ---------------- stage 3: output projection ---------------------------
    def stage3(tt_range, copy_engine="dve", pool=None, last=False):
        for tt in tt_range:
            ot = outsb.tile([P, DM], BF16, tag="ot")
            for oo in range(4):
                if pool is None:
                    po = psum_proj.tile([P, 512], F32, tag="proj")
                elif pool == "alt":
                    if oo % 2 == 0:
                        po = psum_proj.tile([P, 512], F32, tag="proj")
                    else:
                        po = psum_sc.tile([P, 512], F32, tag="sc",
                                          name=f"po{tt}_{oo}")
                else:
                    po = pool.tile([P, 512], F32, tag="sc", name=f"po{tt}_{oo}")
                for h in range(2):
                    nc.tensor.matmul(po, lhsT=attnT[h][:, tt * P:(tt + 1) * P],
                                     rhs=wosb[:, h, oo * 512:(oo + 1) * 512],
                                     start=(h == 0), stop=(h == 1))
                oslice = ot[:, oo * 512:(oo + 1) * 512]
                if last:
                    # drain tail: alternate copy engines and DMA per 1KB-wide
                    # half so copies and output DMAs pipeline instead of
                    # serializing four descriptor setups at the very end
                    if oo % 2 == 0:
                        nc.vector.tensor_copy(out=oslice, in_=po)
                    else:
                        nc.scalar.copy(out=oslice, in_=po)
                        nc.sync.dma_start(
                            out=out[tt * P:(tt + 1) * P,
                                    (oo - 1) * 512:(oo + 1) * 512],
                            in_=ot[:, (oo - 1) * 512:(oo + 1) * 512])
                    continue
                if copy_engine == "dve":
                    nc.vector.tensor_copy(out=oslice, in_=po)
                elif copy_engine == "act":
                    nc.scalar.copy(out=oslice, in_=po)
                else:  # both
                    if oo % 2 == 0:
                        nc.vector.tensor_copy(out=oslice, in_=po)
                    else:
                        nc.scalar.copy(out=oslice, in_=po)
            if not last:
                # one batched DMA per chunk: 4x fewer HWDGE descriptor setups
                nc.sync.dma_start(out=out[tt * P:(tt + 1) * P, :], in_=ot)

    # ---- emission order tuned for overlap ---------------------------------
    # unit pipeline driver: mm(unit k) | consume(unit k-1) | finish(unit k-2)
    mm_pend = []
    cons_pend = []

    def pump(u=None):
        if u is not None:
            mm_pend.append(stage1_mm(*u))
        if mm_pend and (len(mm_pend) >= 2 or u is None):
            cons_pend.append(stage1_consume(mm_pend.pop(0)))
        if cons_pend and (len(cons_pend) >= 2 or u is None):
            stage1_finish(cons_pend.pop(0))

    # startup DMAs in strict first-use order; path-0 units for chunks 0-3 run
    # first (they only need wcat0 + small x pieces), the fading path starts
    # once its kv weight half lands, its q half streams later still
    nc.sync.dma_start(out=bsb[0], in_=b_ap[0])
    nc.sync.dma_start(out=bsb[1], in_=b_ap[1])
    xt0 = xstream.tile([P, ND, P], BF16, tag="xt")
    nc.sync.dma_start(out=xt0[:, 0:4], in_=xTt[0][:, 0:4])
    nc.sync.dma_start(out=wsb[0][:, 0:4], in_=w_ap[0][:, 0:4])
    nc.sync.dma_start(out=xt0[:, 4:], in_=xTt[0][:, 4:])
    nc.sync.dma_start(out=wsb[0][:, 4:8], in_=w_ap[0][:, 4:8])
    nc.sync.dma_start(out=wsb[0][:, 8:], in_=w_ap[0][:, 8:])
    xt_tiles[0] = xt0
    prefetch_xt(1)
    prefetch_tab(0)
    prefetch_xt(2)
    prefetch_tab(1)
    prefetch_xt(3)
    prefetch_tab(2)
    nc.sync.dma_start(out=wsb[1][:, :, 256:], in_=w_ap[1][:, :, 256:])
    prefetch_tab(3)
    prefetch_xt(4)
    prefetch_tab(4)

    pump((0, 0))
    pump((1, 0))
    pump((2, 0))
    nc.sync.dma_start(out=wsb[1][:, :, 0:256], in_=w_ap[1][:, :, 0:256])
    pump((3, 0))          # prefetches chunk 5
    pump((0, 1))
    pump((4, 0))          # prefetches chunk 6
    nc.sync.dma_start(out=wosb, in_=wo)  # off the startup critical path
    for u in [(1, 1), (5, 0), (2, 1), (6, 0), (3, 1), (7, 0), (4, 1),
              (8, 0), (5, 1), (6, 1), (7, 1), (8, 1)]:
        pump(u)
    pump()   # consume (8,1), finish (7,1): batch-0 qkT complete

    # batch-0 attention, software-pipelined with batch-1 projections and the
    # first Wo chunks as dense PE filler between exp-gated score/PV bursts
    exp0_ic = expool.tile([P, 2, NB, 384], BF16, tag="exp_ic")
    exp0_f = expool.tile([P, 2, 6, 768], BF16, tag="exp_f")
    gtr0_t = psum_tr.tile([P, 512], BF16, tag="gtr")
    gtr0 = [gtr0_t[:, h * 256:(h + 1) * 256] for h in range(2)]
    s3_after0 = {4: [0], 5: [1, 2], 6: [3, 4], 7: [5]}
    stage2_scores(0, 0, exp0_ic, exp0_f)
    for bj in range(NB):
        if bj + 1 < NB:
            stage2_scores(0, bj + 1, exp0_ic, exp0_f)
        if bj == 7:
            # drain the last stage1 units before the final PV so batch-1
            # qkT copies overlap batch-0's tail instead of stalling batch 1
            pump()
            pump()
        if bj < 7:
            pump((9 + bj, 0))
        stage2_pv(0, bj, exp0_ic, exp0_f, gtr0)
        if bj < 7:
            pump((9 + bj, 1))
        for tt in s3_after0.get(bj, []):
            stage3([tt], copy_engine="act", pool=psum_sc)

    # batch-1 attention, with its Wo chunks as filler (bi done at bj >= bi)
    exp1_ic = expool.tile([P, 2, NB, 384], BF16, tag="exp_ic")
    exp1_f = expool.tile([P, 2, 6, 768], BF16, tag="exp_f")
    gtr1_t = psum_tr.tile([P, 512], BF16, tag="gtr")
    gtr1 = [gtr1_t[:, h * 256:(h + 1) * 256] for h in range(2)]
    s3_after1 = {0: [6, 7], 1: [8], 2: [9], 3: [10], 4: [11], 5: [12, 13],
                 6: [14], 7: [15]}
    stage2_scores(1, 0, exp1_ic, exp1_f)
    for bj in range(NB):
        if bj + 1 < NB:
            stage2_scores(1, bj + 1, exp1_ic, exp1_f)
        stage2_pv(1, bj, exp1_ic, exp1_f, gtr1, solo=(bj >= 6))
        for tt in s3_after1.get(bj, []):
            stage3([tt],
                   copy_engine=("dve" if bj <= 1 else "both"),
                   pool=(psum_sc if bj == 7 else None),
                   last=(tt >= NT - 2))

    for pool in reversed((consts, weights, resident, xstream, tstream, work,
                          qpipe, stats, expool, outsb, psum_proj, psum_sc,
                          psum_pv, psum_tr)):
        pool.release()


_NC_CACHE = {}


def _get_nc():
    if "nc" not in _NC_CACHE:
        nc = bacc.Bacc("TRN2", target_bir_lowering=False, debug=False,
                       num_devices=N_CORES)
        with tile.TileContext(nc) as tc:
            _build_tile_kernel(tc)
        nc.compile()
        _NC_CACHE["nc"] = nc
    return _NC_CACHE["nc"]


def _prep_in_maps(inputs):
    f32 = np.float32
    x = np.asarray(inputs["hidden_states"], f32).reshape(T, DM)
    cos = np.asarray(inputs["cos"], f32).reshape(T, D)[:S]
    sin = np.asarray(inputs["sin"], f32).reshape(T, D)[:S]

    xT = np.ascontiguousarray(x.T)
    xTt = np.ascontiguousarray(
        xT.reshape(ND, P, NT, P).transpose(2, 1, 0, 3)).astype(BFNP)

    sign = np.concatenate([-np.ones(64, f32), np.ones(64, f32)])
    A = D ** 0.25   # a*b = SCALE*D split evenly between the q and k tables

    def fold(g):
        g = np.asarray(g, f32)
        cg = cos * (A * g)[None, :]
        sg = sin * (A * sign * np.concatenate([g[64:], g[:64]]))[None, :]
        return cg, sg

    # tabs identical for every core (gammas are global) and both batches
    tabs = np.empty((S, 2, 2, 384), f32)
    for p, (gq_name, gk_name) in enumerate([("gq", "gk"), ("gq2", "gk2")]):
        cgq, sgq = fold(inputs[gq_name])
        cgk, sgk = fold(inputs[gk_name])
        tabs[:, p, 0, :] = np.concatenate([cgq, cgq, cgk], 1)
        tabs[:, p, 1, :] = np.concatenate([sgq, sgq, sgk], 1)
    tabs = tabs.reshape(NB, P, 2, 2, 384).astype(BFNP)

    Wo = np.asarray(inputs["Wo"], f32)

    in_maps = []
    for c in range(N_CORES):
        m = {"xTt": xTt, "tabs": tabs}
        for p, names in enumerate([("Wq", "bq", "Wk", "bk", "Wv", "bv"),
                                   ("Wq2", "bq2", "Wk2", "bk2", "Wv2", "bv2")]):
            Wq, bq, Wk, bk, Wv, bv = (np.asarray(inputs[n], f32) for n in names)
            Wcat = np.concatenate([Wq[c * 256:(c + 1) * 256],
                                   Wk[c * P:(c + 1) * P],
                                   Wv[c * P:(c + 1) * P]], 0)      # [512, DM]
            wcatT = np.ascontiguousarray(Wcat.T)                    # [DM, 512]
            m[f"wcat{p}"] = np.ascontiguousarray(
                wcatT.reshape(ND, P, 512).transpose(1, 0, 2)).astype(BFNP)
            bcat = np.concatenate([bq[c * 256:(c + 1) * 256],
                                   bk[c * P:(c + 1) * P],
                                   bv[c * P:(c + 1) * P]])
            m[f"bcat{p}"] = bcat.reshape(1, 512).astype(BFNP)
        woT = np.ascontiguousarray(Wo[:, c * 256:(c + 1) * 256].T)  # [256, DM]
        m["woT"] = np.ascontiguousarray(
            woT.reshape(2, P, DM).transpose(1, 0, 2)).astype(BFNP)
        in_maps.append(m)
    return in_maps


def kernel(**inputs) -> np.ndarray:
    nc = _get_nc()
    in_maps = _prep_in_maps(inputs)
    res = bass_utils.run_bass_kernel_spmd(nc, in_maps, core_ids=list(range(N_CORES)))
    total = np.zeros((T, DM), np.float32)
    for c in range(N_CORES):
        total += res.results[c]["out"].astype(np.float32)
    return total.reshape(B, S, DM)


# revision 66
# speedup vs baseline: 1.1477x; 1.0649x over previous
"""BMOJO attention (sliding-window + fading memory, joint softmax) on 8 TRN2
NeuronCores via Bass/Tile.

Sharding: tensor-parallel over heads — core c owns q-heads {2c, 2c+1} and kv
head c for both batches and both projection paths; each core computes a partial
output through its Wo column shard and the host sums the 8 partials.

Math (per core, all matmuls bf16 with fp32 PSUM accumulation):
  1. qkv = x @ Wcat.T + bcat  for both paths (Wcat = [Wq_sh; Wk_sh; Wv_sh])
  2. rmsnorm scales r = 1/sqrt(ssq) computed as exp(-0.5*ln(ssq)) so the whole
     kernel uses a single activation table (ln/exp/square/copy); the D**0.25
     rmsnorm/softmax constants are folded into the host-side rope tables
     (cg = cos*g*A, sg = sign*shift(g)*sin*A); rq applied to q, rk folded into
     the exp() scale of the score pass.
  3. scores computed transposed sT[j, i] = k~ @ q~.T so the softmax exp tiles
     feed the PV matmul as the stationary operand without any p-transpose.
     Max-free softmax: p = exp(rk*s), 0/1 block masks after exp.
  4. PV in [i, e] with a ones-column appended to V: the PSUM accumulator picks
     up the joint (in-window + fading) softmax denominator for free.
  5. attn normalized, PE-transposed, then attnT @ WoT_shard -> partial out.

Schedule: stage1 is software-pipelined (chunk tt's PE transposes are emitted
after chunk tt+1's projection matmuls, hiding the ~2.5us rmsnorm/rope
stats latency); batch-0 attention interleaves the remaining batch-1 stage1
chunks and early Wo chunks as PE filler; batch-1 attention interleaves the
rest of the output projection.
"""
import numpy as np
import ml_dtypes

import concourse.bass as bass
import concourse.tile as tile
from concourse import bacc, mybir
from concourse import bass_utils
from concourse.masks import make_identity

BFNP = ml_dtypes.bfloat16
F32 = mybir.dt.float32
BF16 = mybir.dt.bfloat16

B, S, DM = 2, 1024, 2048
H, HKV, D = 16, 8, 128
W = 256
SCALE = D ** -0.5
P = 128
T = B * S           # 2048 flattened tokens
NT = T // P         # 16 t-chunks
ND = DM // P        # 16 d-chunks
NB = S // P         # 8 s-blocks per batch
N_CORES = 8

AluOp = mybir.AluOpType
ACT_FN = mybir.ActivationFunctionType


def _ic_width(bj):
    # in-window scores for key block bj cover query blocks {bj, bj+1, bj+2}
    return min(P * (bj + 3), S) - P * bj


def _f_width(bj):
    # fading scores for key block bj cover query blocks {bj+2 .. NB-1}
    return max(0, S - P * (bj + 2))


def _p1c(tt):
    # dead fading-path columns: its q is never used by queries i < W
    # (s-blocks 0,1) and its k/v never serve keys j > S-W (s-blocks 6,7)
    sblk = tt % NB
    return (256, 512) if sblk <= 1 else (0, 256) if sblk >= NB - 2 \
        else (0, 512)


def _build_tile_kernel(tc):
    nc = tc.nc

    xTt = nc.dram_tensor("xTt", (NT, P, ND, P), BF16, kind="ExternalInput").ap()
    w_ap = [
        nc.dram_tensor(f"wcat{p}", (P, ND, 512), BF16, kind="ExternalInput").ap()
        for p in range(2)
    ]
    b_ap = [
        nc.dram_tensor(f"bcat{p}", (1, 512), BF16, kind="ExternalInput").ap()
        for p in range(2)
    ]
    # rope tables: [sblk, p, path, cg/sg, 384]; identical for both batches
    tabs = nc.dram_tensor("tabs", (NB, P, 2, 2, 384), BF16, kind="ExternalInput").ap()
    wo = nc.dram_tensor("woT", (P, 2, DM), BF16, kind="ExternalInput").ap()
    out = nc.dram_tensor("out", (T, DM), BF16, kind="ExternalOutput").ap()

    consts = tc.alloc_tile_pool(name="consts", bufs=1)
    weights = tc.alloc_tile_pool(name="weights", bufs=1)
    resident = tc.alloc_tile_pool(name="resident", bufs=1)
    xstream = tc.alloc_tile_pool(name="xstream", bufs=7)
    tstream = tc.alloc_tile_pool(name="tstream", bufs=7)
    work = tc.alloc_tile_pool(name="work", bufs=7)
    qpipe = tc.alloc_tile_pool(name="qpipe", bufs=6)
    stats = tc.alloc_tile_pool(name="stats", bufs=4)
    expool = tc.alloc_tile_pool(name="expool", bufs=1)
    outsb = tc.alloc_tile_pool(name="outsb", bufs=3)
    psum_proj = tc.alloc_tile_pool(name="psum_proj", bufs=2, space="PSUM")
    psum_sc = tc.alloc_tile_pool(name="psum_sc", bufs=3, space="PSUM")
    psum_pv = tc.alloc_tile_pool(name="psum_pv", bufs=2, space="PSUM")
    psum_tr = tc.alloc_tile_pool(name="psum_tr", bufs=1, space="PSUM")

    # constants
    ident = consts.tile([P, P], BF16)
    make_identity(nc, ident)
    t1m = consts.tile([P, P], BF16)   # keep i' >= j'  (partition = j', free = i')
    nc.gpsimd.memset(t1m, 1.0)
    nc.gpsimd.affine_select(out=t1m, in_=t1m, compare_op=AluOp.is_ge, fill=0.0,
                            base=0, pattern=[[1, P]], channel_multiplier=-1)
    t2m = consts.tile([P, P], BF16)   # keep i' < j'  i.e. (j' - i' - 1) >= 0
    nc.gpsimd.memset(t2m, 1.0)
    nc.gpsimd.affine_select(out=t2m, in_=t2m, compare_op=AluOp.is_ge, fill=0.0,
                            base=-1, pattern=[[-1, P]], channel_multiplier=1)
    ones1 = consts.tile([1, P], BF16)
    nc.vector.memset(ones1, 1.0)

    # big resident inputs
    wsb = [weights.tile([P, ND, 512], BF16, name=f"wsb{p}") for p in range(2)]
    bsb = [weights.tile([1, 512], BF16, name=f"bsb{p}") for p in range(2)]
    wosb = weights.tile([P, 2, DM], BF16)

    # per-path residents: qkT[path]: [d=128, slot(q0,q1,k), t], v(+ones)
    qkT = [resident.tile([P, 3, T], BF16, name=f"qkT{p}") for p in range(2)]
    vsb = [resident.tile([P, NT, P + 1], BF16, name=f"vsb{p}") for p in range(2)]
    # rall[:, tt, p*3:p*3+3] = (rq0, rq1, rk) = 1/sqrt(ssq) for chunk tt, path p
    rall = resident.tile([P, NT, 6], F32, name="rall")
    for p in range(2):
        nc.vector.memset(vsb[p][:, :, P:P + 1], 1.0)
    attnT = [resident.tile([P, T], BF16, name=f"attnT{h}") for h in range(2)]

    # ---------------- stage 1 (pipelined): mm / consume / finish ------------
    xt_tiles = {}
    tab_tiles = {}

    def prefetch_xt(tt, split=False):
        # split: two pieces so the first d-chunk matmuls can start while the
        # second half is still in flight
        xt = xstream.tile([P, ND, P], BF16, tag="xt")
        if split:
            nc.sync.dma_start(out=xt[:, 0:8], in_=xTt[tt][:, 0:8])
            nc.sync.dma_start(out=xt[:, 8:], in_=xTt[tt][:, 8:])
        else:
            nc.sync.dma_start(out=xt, in_=xTt[tt])
        xt_tiles[tt] = xt

    def prefetch_tab(tt):
        tab = tstream.tile([P, 2, 2, 384], BF16, tag="tab")
        nc.sync.dma_start(out=tab, in_=tabs[tt % NB])
        tab_tiles[tt] = tab

    def prefetch(tt):
        if tt >= NT or tt in xt_tiles:
            return
        prefetch_xt(tt)
        prefetch_tab(tt)

    def stage1_mm(tt, p):
        if p == 0:
            prefetch(tt + 2)
        xt = xt_tiles[tt]
        tab = tab_tiles[tt]
        p1c = _p1c(tt)
        c0, c1 = (0, 512) if p == 0 else p1c
        ps = psum_proj.tile([P, 512], F32, tag="proj")
        # bias via K=1 matmul, then accumulate the 16 d-chunks
        nc.tensor.matmul(ps[:, c0:c1], lhsT=ones1, rhs=bsb[p][:, c0:c1],
                         start=True, stop=False)
        for dd in range(ND):
            nc.tensor.matmul(ps[:, c0:c1], lhsT=xt[:, dd, :],
                             rhs=wsb[p][:, dd, c0:c1],
                             start=False, stop=(dd == ND - 1))
        if p == 1:
            xt_tiles.pop(tt)
        return (tt, p, p1c, ps, tab)

    def stage1_consume(mm_ctx):
        # per-(chunk, path) unit: square/v-copy (Act) and rope (DVE) free the
        # proj psum early; the reduce/Newton/til chain has two whole mm-phases
        # of slack before stage1_finish needs til
        tt, p, p1c, ps, tab = mm_ctx
        c0, sc1 = (0, 384) if p == 0 else (p1c[0], min(p1c[1], 384))
        w = sc1 - c0
        s0, s1 = c0 // P, sc1 // P
        # squares (one ACT op; DVE can't — walrus allows only one PSUM
        # input per instruction)
        sqsb = work.tile([P, 3, P], BF16, tag="sqsb")
        nc.scalar.activation(
            out=sqsb[:, s0:s1, :].rearrange("p a b -> p (a b)"),
            in_=ps[:, c0:sc1], func=ACT_FN.Square)
        # v (+ ones col already set)
        if p == 0 or p1c[1] == 512:
            nc.scalar.copy(out=vsb[p][:, tt, 0:P], in_=ps[:, 384:512])

        # rope over the live head-slots at once
        cg = tab[:, p, 0, c0:sc1]
        sg = tab[:, p, 1, c0:sc1]
        ra = work.tile([P, 384], BF16, tag="ra")
        nc.vector.tensor_tensor(out=ra[:, c0:sc1], in0=ps[:, c0:sc1],
                                in1=cg, op=AluOp.mult)
        # rotate-half read of the psum q/k: one op via a reversed-half AP
        psw = ps[:, c0:sc1]
        pr_sw = bass.AP(tensor=psw.tensor, offset=psw.offset + 64,
                        ap=[list(psw.ap[0]), [128, w // P], [-64, 2],
                            [1, 64]])
        rb = work.tile([P, 384], BF16, tag="rb")
        nc.vector.tensor_tensor(
            out=rb[:, c0:sc1].rearrange("p (h s d) -> p h s d",
                                        h=w // P, s=2, d=64),
            in0=pr_sw,
            in1=sg.rearrange("p (h s d) -> p h s d", h=w // P, s=2, d=64),
            op=AluOp.mult)
        qkn = qpipe.tile([P, 384], BF16, tag="qkn")
        nc.vector.tensor_add(out=qkn[:, c0:sc1], in0=ra[:, c0:sc1],
                             in1=rb[:, c0:sc1])

        # per-head-slot sums (one DVE reduce), then r = 1/sqrt(ssq) via
        # bit-trick + 2 Newton steps, all on DVE ALUs so the Activation
        # engine only ever needs one function table (exp); the D**0.25
        # constants live in the host-folded rope tables and eps is
        # negligible (ssq ~ D >> eps). MAGIC - (x>>1) is computed as
        # ((x>>1) ^ -1) + (MAGIC+1) to avoid a reversed subtract.
        ssq3 = stats.tile([P, 3], F32, tag="ssq3")
        yc = stats.tile([P, 3], F32, tag="yc")
        nt = stats.tile([P, 3], F32, tag="nt")
        xi = ssq3.bitcast(mybir.dt.int32)
        yi = yc.bitcast(mybir.dt.int32)
        nc.vector.tensor_reduce(out=ssq3[:, s0:s1], in_=sqsb[:, s0:s1, :],
                                axis=mybir.AxisListType.X, op=AluOp.add)
        nc.vector.tensor_scalar(out=yi[:, s0:s1], in0=xi[:, s0:s1],
                                scalar1=1, scalar2=-1,
                                op0=AluOp.arith_shift_right,
                                op1=AluOp.bitwise_xor)
        nc.vector.tensor_scalar(out=yi[:, s0:s1], in0=yi[:, s0:s1],
                                scalar1=0x5f3759df + 1, scalar2=None,
                                op0=AluOp.add)
        for it in range(2):
            dst = yc[:, s0:s1] if it == 0 \
                else rall[:, tt, p * 3 + s0:p * 3 + s1]
            nc.vector.tensor_tensor(out=nt[:, s0:s1], in0=yc[:, s0:s1],
                                    in1=yc[:, s0:s1], op=AluOp.mult)
            nc.vector.tensor_tensor(out=nt[:, s0:s1], in0=nt[:, s0:s1],
                                    in1=ssq3[:, s0:s1], op=AluOp.mult)
            nc.vector.tensor_scalar(out=nt[:, s0:s1], in0=nt[:, s0:s1],
                                    scalar1=-0.5, scalar2=1.5,
                                    op0=AluOp.mult, op1=AluOp.add)
            nc.vector.tensor_tensor(out=dst, in0=yc[:, s0:s1],
                                    in1=nt[:, s0:s1], op=AluOp.mult)

        til = None
        if s0 == 0:
            # rq scaling on the Act engine (copy with per-partition scale) —
            # DVE is the tight engine during stage1
            til = qpipe.tile([P, 256], BF16, tag="til")
            for h in range(2):
                nc.scalar.activation(
                    out=til[:, h * P:(h + 1) * P],
                    in_=qkn[:, h * P:(h + 1) * P], func=ACT_FN.Copy,
                    scale=rall[:, tt, p * 3 + h:p * 3 + h + 1])
        return (tt, p, qkn, til, c0, sc1)

    def stage1_finish(cons_ctx):
        # emitted two units behind the projection matmuls so the PE-side
        # transposes never wait on the rmsnorm/rope stats chain
        tt, p, qkn, til, c0, sc1 = cons_ctx
        w = sc1 - c0
        s0, s1 = c0 // P, sc1 // P
        tr = psum_pv.tile([P, 384], BF16, tag="pv", name=f"tr{tt}_{p}")
        if s0 == 0:
            nc.tensor.transpose(tr[:, 0:P], til[:, 0:P], ident)
            nc.tensor.transpose(tr[:, P:2 * P], til[:, P:2 * P], ident)
        if s1 == 3:
            nc.tensor.transpose(tr[:, 2 * P:3 * P], qkn[:, 2 * P:3 * P], ident)
        nc.vector.tensor_copy(
            out=qkT[p][:, s0:s1, tt * P:(tt + 1) * P],
            in_=tr[:, c0:sc1].rearrange("p (h t) -> p h t", h=w // P))

    # ---------------- stage 2: attention, both heads of one batch ----------
    # block-level software pipeline: scores/exp/diag-mask of block bj+1 are
    # emitted before the PVs of block bj, so a PV's exp tiles are always a
    # full block-slot old when the PE reaches them
    def stage2_scores(b, bj, exp_ic, exp_f):
        kt_ic = qkT[0][:, 2, (b * S + bj * P):(b * S + (bj + 1) * P)]
        kt_f = qkT[1][:, 2, (b * S + bj * P):(b * S + (bj + 1) * P)]
        w_ic = _ic_width(bj)
        i0 = b * S + bj * P
        wf = _f_width(bj)
        i0f = b * S + P * (bj + 2)
        for h in range(2):
            pssc = psum_sc.tile([P, 512], F32, tag="sc")
            nc.tensor.matmul(pssc[:, 0:w_ic], lhsT=kt_ic,
                             rhs=qkT[0][:, h, i0:i0 + w_ic],
                             start=True, stop=True)
            nc.scalar.activation(out=exp_ic[:, h, bj, 0:w_ic],
                                 in_=pssc[:, 0:w_ic], func=ACT_FN.Exp,
                                 scale=rall[:, b * NB + bj, 2:3])
            for c0 in range(0, wf, 512):
                wc = min(512, wf - c0)
                psf = psum_sc.tile([P, 512], F32, tag="sc")
                nc.tensor.matmul(psf[:, 0:wc], lhsT=kt_f,
                                 rhs=qkT[1][:, h, i0f + c0:i0f + c0 + wc],
                                 start=True, stop=True)
                nc.scalar.activation(
                    out=exp_f[:, h, bj, c0:c0 + wc], in_=psf[:, 0:wc],
                    func=ACT_FN.Exp, scale=rall[:, b * NB + bj, 5:6])
            # only the diagonal mask gates this block's own PV — emit it
            # immediately; the other masks are needed two blocks later
            dia = exp_ic[:, h, bj, 0:P]
            nc.gpsimd.tensor_tensor(out=dia, in0=dia, in1=t1m, op=AluOp.mult)

    def stage2_pv(b, bj, exp_ic, exp_f, group_tr, solo=False):
        w_ic = _ic_width(bj)
        wf = _f_width(bj)
        # PV for query block bi == bj; diagonal (freshest exp) last
        bi = bj
        for h in range(2):
            pv = psum_pv.tile([P, P + 1], F32, tag="pv")
            mms = []
            for bjj in range(0, bi - 1):
                mms.append((exp_f[:, h, bjj, (bi - bjj - 2) * P:(bi - bjj - 1) * P],
                            vsb[1][:, b * NB + bjj, :]))
            for bjj in range(max(0, bi - 2), bi):
                mms.append((exp_ic[:, h, bjj, (bi - bjj) * P:(bi - bjj + 1) * P],
                            vsb[0][:, b * NB + bjj, :]))
            mms.append((exp_ic[:, h, bi, 0:P], vsb[0][:, b * NB + bi, :]))
            for mi, (lhsT, rhs) in enumerate(mms):
                nc.tensor.matmul(pv, lhsT=lhsT, rhs=rhs,
                                 start=(mi == 0), stop=(mi == len(mms) - 1))
            rl = stats.tile([P, 1], F32, tag="rl")
            nc.vector.reciprocal(rl, pv[:, P:P + 1])
            anorm = work.tile([P, P], BF16, tag="anorm")
            nc.vector.tensor_scalar_mul(out=anorm, in0=pv[:, 0:P], scalar1=rl)
            # pair up transposed blocks per psum bank; one copy per pair, so
            # attnT[2b:2b+2] is available to stage3 right after block 2b+1.
            # solo: per-block copy so the final Wo chunks start a block early
            if solo:
                nc.tensor.transpose(group_tr[h][:, 0:P], anorm, ident)
                nc.vector.tensor_copy(
                    out=attnT[h][:, (b * S + bi * P):(b * S + (bi + 1) * P)],
                    in_=group_tr[h][:, 0:P])
                continue
            nc.tensor.transpose(
                group_tr[h][:, (bi % 2) * P:(bi % 2 + 1) * P], anorm, ident)
            if bi % 2 == 1:
                t0 = b * S + (bi - 1) * P
                nc.vector.tensor_copy(out=attnT[h][:, t0:t0 + 256],
                                      in_=group_tr[h])

        # deferred masks (consumed by PV of block bj+2)
        for h in range(2):
            if w_ic > 256:
                ic2 = exp_ic[:, h, bj, 256:384]
                nc.gpsimd.tensor_tensor(out=ic2, in0=ic2, in1=t2m,
                                        op=AluOp.mult)
            if wf > 0:
                f2 = exp_f[:, h, bj, 0:P]
                nc.gpsimd.tensor_tensor(out=f2, in0=f2, in1=t1m,
                                        op=AluOp.mult)

    # ---------------- stage 3: output projection ---------------------------
    def stage3(tt_range, copy_engine="dve", pool=None, last=False):
        for tt in tt_range:
            ot = outsb.tile([P, DM], BF16, tag="ot")
            for oo in range(4):
                if pool is None:
                    po = psum_proj.tile([P, 512], F32, tag="proj")
                elif pool == "alt":
                    if oo % 2 == 0:
                        po = psum_proj.tile([P, 512], F32, tag="proj")
                    else:
                        po = psum_sc.tile([P, 512], F32, tag="sc",
                                          name=f"po{tt}_{oo}")
                else:
                    po = pool.tile([P, 512], F32, tag="sc", name=f"po{tt}_{oo}")
                for h in range(2):
                    nc.tensor.matmul(po, lhsT=attnT[h][:, tt * P:(tt + 1) * P],
                                     rhs=wosb[:, h, oo * 512:(oo + 1) * 512],
                                     start=(h == 0), stop=(h == 1))
                oslice = ot[:, oo * 512:(oo + 1) * 512]
                if last:
                    # drain tail: alternate copy engines and DMA per 1KB-wide
                    # half so copies and output DMAs pipeline instead of
                    # serializing four descriptor setups at the very end
                    if oo % 2 == 0:
                        nc.vector.tensor_copy(out=oslice, in_=po)
                    else:
                        nc.scalar.copy(out=oslice, in_=po)
                        nc.sync.dma_start(
                            out=out[tt * P:(tt + 1) * P,
                                    (oo - 1) * 512:(oo + 1) * 512],
                            in_=ot[:, (oo - 1) * 512:(oo + 1) * 512])
                    continue
                if copy_engine == "dve":
                    nc.vector.tensor_copy(out=oslice, in_=po)
                elif copy_engine == "act":
                    nc.scalar.copy(out=oslice, in_=po)
                else:  # both
                    if oo % 2 == 0:
                        nc.vector.tensor_copy(out=oslice, in_=po)
                    else:
                        nc.scalar.copy(out=oslice, in_=po)
            if not last:
                # one batched DMA per chunk: 4x fewer HWDGE descriptor setups
                nc.sync.dma_start(out=out[tt * P:(tt + 1) * P, :], in_=ot)

    # ---- emission order tuned for overlap ---------------------------------
    # unit pipeline driver: mm(unit k) | consume(unit k-1) | finish(unit k-2)
    mm_pend = []
    cons_pend = []

    def pump(u=None):
        if u is not None:
            mm_pend.append(stage1_mm(*u))
        if mm_pend and (len(mm_pend) >= 2 or u is None):
            cons_pend.append(stage1_consume(mm_pend.pop(0)))
        if cons_pend and (len(cons_pend) >= 2 or u is None):
            stage1_finish(cons_pend.pop(0))

    # startup DMAs in strict first-use order; path-0 units for chunks 0-3 run
    # first (they only need wcat0 + small x pieces), the fading path starts
    # once its kv weight half lands, its q half streams later still
    nc.sync.dma_start(out=bsb[0], in_=b_ap[0])
    nc.sync.dma_start(out=bsb[1], in_=b_ap[1])
    xt0 = xstream.tile([P, ND, P], BF16, tag="xt")
    nc.sync.dma_start(out=xt0[:, 0:4], in_=xTt[0][:, 0:4])
    nc.sync.dma_start(out=wsb[0][:, 0:4], in_=w_ap[0][:, 0:4])
    nc.sync.dma_start(out=xt0[:, 4:], in_=xTt[0][:, 4:])
    nc.sync.dma_start(out=wsb[0][:, 4:8], in_=w_ap[0][:, 4:8])
    nc.sync.dma_start(out=wsb[0][:, 8:], in_=w_ap[0][:, 8:])
    xt_tiles[0] = xt0
    prefetch_xt(1)
    prefetch_tab(0)
    prefetch_xt(2)
    prefetch_tab(1)
    prefetch_xt(3)
    prefetch_tab(2)
    nc.sync.dma_start(out=wsb[1][:, :, 256:], in_=w_ap[1][:, :, 256:])
    prefetch_tab(3)
    prefetch_xt(4)
    prefetch_tab(4)

    pump((0, 0))
    pump((1, 0))
    pump((2, 0))
    nc.sync.dma_start(out=wsb[1][:, :, 0:256], in_=w_ap[1][:, :, 0:256])
    pump((3, 0))          # prefetches chunk 5
    pump((0, 1))
    pump((4, 0))          # prefetches chunk 6
    nc.sync.dma_start(out=wosb, in_=wo)  # off the startup critical path
    for u in [(1, 1), (5, 0), (2, 1), (6, 0), (3, 1), (7, 0), (4, 1),
              (8, 0), (5, 1), (6, 1), (7, 1), (8, 1)]:
        pump(u)
    pump()   # consume (8,1), finish (7,1): batch-0 qkT complete

    # batch-0 attention, software-pipelined with batch-1 projections and the
    # first Wo chunks as dense PE filler between exp-gated score/PV bursts
    exp0_ic = expool.tile([P, 2, NB, 384], BF16, tag="exp_ic")
    exp0_f = expool.tile([P, 2, 6, 768], BF16, tag="exp_f")
    gtr0_t = psum_tr.tile([P, 512], BF16, tag="gtr")
    gtr0 = [gtr0_t[:, h * 256:(h + 1) * 256] for h in range(2)]
    s3_after0 = {4: [0], 5: [1, 2], 6: [3, 4], 7: [5]}
    stage2_scores(0, 0, exp0_ic, exp0_f)
    for bj in range(NB):
        if bj + 1 < NB:
            stage2_scores(0, bj + 1, exp0_ic, exp0_f)
        if bj == 7:
            # drain the last stage1 units before the final PV so batch-1
            # qkT copies overlap batch-0's tail instead of stalling batch 1
            pump()
            pump()
        if bj < 7:
            pump((9 + bj, 0))
        stage2_pv(0, bj, exp0_ic, exp0_f, gtr0)
        if bj < 7:
            pump((9 + bj, 1))
        for tt in s3_after0.get(bj, []):
            stage3([tt], copy_engine="act", pool=psum_sc)

    # batch-1 attention, with its Wo chunks as filler (bi done at bj >= bi)
    exp1_ic = expool.tile([P, 2, NB, 384], BF16, tag="exp_ic")
    exp1_f = expool.tile([P, 2, 6, 768], BF16, tag="exp_f")
    gtr1_t = psum_tr.tile([P, 512], BF16, tag="gtr")
    gtr1 = [gtr1_t[:, h * 256:(h + 1) * 256] for h in range(2)]
    s3_after1 = {0: [6, 7], 1: [8], 2: [9], 3: [10], 4: [11], 5: [12, 13],
                 6: [14], 7: [15]}
    stage2_scores(1, 0, exp1_ic, exp1_f)
    for bj in range(NB):
        if bj + 1 < NB:
            stage2_scores(1, bj + 1, exp1_ic, exp1_f)
        stage2_pv(1, bj, exp1_ic, exp1_f, gtr1, solo=(bj >= 6))
        for tt in s3_after1.get(bj, []):
            stage3([tt],
                   copy_engine=("dve" if bj <= 1 else "both"),
                   pool=(psum_sc if bj == 7 else None),
                   last=(tt >= NT - 2))

    for pool in reversed((consts, weights, resident, xstream, tstream, work,
                          qpipe, stats, expool, outsb, psum_proj, psum_sc,
                          psum_pv, psum_tr)):
        pool.release()


_NC_CACHE = {}


def _get_nc():
    if "nc" not in _NC_CACHE:
        nc = bacc.Bacc("TRN2", target_bir_lowering=False, debug=False,
                       num_devices=N_CORES)
        with tile.TileContext(nc) as tc:
            _build_tile_kernel(tc)
        nc.compile()
        _NC_CACHE["nc"] = nc
    return _NC_CACHE["nc"]


def _prep_in_maps(inputs):
    f32 = np.float32
    x = np.asarray(inputs["hidden_states"], f32).reshape(T, DM)
    cos = np.asarray(inputs["cos"], f32).reshape(T, D)[:S]
    sin = np.asarray(inputs["sin"], f32).reshape(T, D)[:S]

    xT = np.ascontiguousarray(x.T)
    xTt = np.ascontiguousarray(
        xT.reshape(ND, P, NT, P).transpose(2, 1, 0, 3)).astype(BFNP)

    sign = np.concatenate([-np.ones(64, f32), np.ones(64, f32)])
    A = D ** 0.25   # a*b = SCALE*D split evenly between the q and k tables

    def fold(g):
        g = np.asarray(g, f32)
        cg = cos * (A * g)[None, :]
        sg = sin * (A * sign * np.concatenate([g[64:], g[:64]]))[None, :]
        return cg, sg

    # tabs identical for every core (gammas are global) and both batches
    tabs = np.empty((S, 2, 2, 384), f32)
    for p, (gq_name, gk_name) in enumerate([("gq", "gk"), ("gq2", "gk2")]):
        cgq, sgq = fold(inputs[gq_name])
        cgk, sgk = fold(inputs[gk_name])
        tabs[:, p, 0, :] = np.concatenate([cgq, cgq, cgk], 1)
        tabs[:, p, 1, :] = np.concatenate([sgq, sgq, sgk], 1)
    tabs = tabs.reshape(NB, P, 2, 2, 384).astype(BFNP)

    Wo = np.asarray(inputs["Wo"], f32)

    in_maps = []
    for c in range(N_CORES):
        m = {"xTt": xTt, "tabs": tabs}
        for p, names in enumerate([("Wq", "bq", "Wk", "bk", "Wv", "bv"),
                                   ("Wq2", "bq2", "Wk2", "bk2", "Wv2", "bv2")]):
            Wq, bq, Wk, bk, Wv, bv = (np.asarray(inputs[n], f32) for n in names)
            Wcat = np.concatenate([Wq[c * 256:(c + 1) * 256],
                                   Wk[c * P:(c + 1) * P],
                                   Wv[c * P:(c + 1) * P]], 0)      # [512, DM]
            wcatT = np.ascontiguousarray(Wcat.T)                    # [DM, 512]
            m[f"wcat{p}"] = np.ascontiguousarray(
                wcatT.reshape(ND, P, 512).transpose(1, 0, 2)).astype(BFNP)
            bcat = np.concatenate([bq[c * 256:(c + 1) * 256],
                                   bk[c * P:(c + 1) * P],
                                   bv[c * P:(c + 1) * P]])
            m[f"bcat{p}"] = bcat.reshape(1, 512).astype(BFNP)
        woT = np.ascontiguousarray(Wo[:, c * 256:(c + 1) * 256].T)  # [256, DM]
        m["woT"] = np.ascontiguousarray(
            woT.reshape(2, P, DM).transpose(1, 0, 2)).astype(BFNP)
        in_maps.append(m)
    return in_maps


def kernel(**inputs) -> np.ndarray:
    nc = _get_nc()
    in_maps = _prep_in_maps(inputs)
    res = bass_utils.run_bass_kernel_spmd(nc, in_maps, core_ids=list(range(N_CORES)))
    total = np.zeros((T, DM), np.float32)
    for c in range(N_CORES):
        total += res.results[c]["out"].astype(np.float32)
    return total.reshape(B, S, DM)
